# revision 14
# baseline (speedup 1.0000x reference)
"""Trainium2 Bass kernel for BuNN (nn_BuNN_10797547782311).

Row-shard L over 8 cores. Per Taylor step, each core contracts its
[2048, 16384] L block (fp8 e3m4, x128 scale, pre-transposed/pre-tiled on
host) against the replicated bf16 term. Key optimizations over v1:

- L stored as fp8 e3m4 (x128): halves HBM traffic; PE-exact (probed).
- 14 of 32 contraction units resident in SBUF (~112KB/partition):
  per-step streamed DMA drops to 18MB.
- Column-tiled matmul pairs (tile_position (0,0)/(0,64)): two 64-wide
  matmuls run concurrently on both halves of the PE array.
- Two-phase steps: output node-blocks {0,2} complete first, are cast /
  transposed / AllGathered while blocks {1,3} compute -> 2 AGs per step
  whose ~24us latency hides under the opposite phase's matmuls.
- Work tiles folded to [128, 1024]: partitions = td x node-half.

Node-parallel math identical to v1 (td-permuted rotation layout, signed
duplicated angles with sin(x + pi/2) for cos).
"""

import os
import sys
import types

import numpy as np
import ml_dtypes

import concourse.bacc as bacc
import concourse.tile as tile
from concourse import mybir
from concourse.bass_utils import run_bass_kernel_spmd
from concourse.bass import ds
from concourse.masks import make_identity

# Problem config (hardcoded)
N, D_IN, D_OUT = 16384, 128, 40
B = 32
TD = 2 * B          # 64
NL = 4              # layers
K = 8               # Taylor steps
M = 8               # cores
R = N // M          # 2048 rows per core

NRES = 13           # resident contraction units (of 32)
NSTR = 32 - NRES    # streamed units
LSC = 128.0         # L fp8 scale (power of 2)
DSC = 1.0 / LSC     # descale for psum -> term

f32 = mybir.dt.float32
bf16 = mybir.dt.bfloat16
fp8 = mybir.dt.float8e3
BF = ml_dtypes.bfloat16
F8 = ml_dtypes.float8_e3m4

# contraction-unit consumption order: (chunk, absolute rank slot j).
# All 8 slots (self included) are consumed from the AllGather output at
# static offsets -- no dynamic-offset DMAs anywhere.
ORDER = [(c, j) for c in (0, 2, 1, 3) for j in range(M)]

# unit-index groups (into ORDER): c0=0..7, c2=8..15, c1=16..23, c3=24..31
GC0 = list(range(0, 8))
GC2 = list(range(8, 16))
GC1 = list(range(16, 24))
GC3 = list(range(24, 32))
# per-step global emission order (phase, units): AG_B-dependent work last
GROUPS_PRE = [(0, GC0), (1, GC0), (0, GC2), (1, GC2)]
GROUPS_A_TAIL = [(0, GC1), (0, GC3)]
GROUPS_B_TAIL = [(1, GC1), (1, GC3)]
STREAM_ORDER = [
    (ph, t)
    for (ph, grp) in GROUPS_PRE + GROUPS_A_TAIL + GROUPS_B_TAIL
    for t in grp
    if t >= NRES
]

_CACHE = {}


def _install_ntff_shim():
    try:
        from antenv.axon_hooks import get_axon_ntff_profile_hook  # noqa: F401
    except ImportError:
        try:
            from trn_agent_boot.trn_boot import _ntff_profile_via_ctypes

            _hook = _ntff_profile_via_ctypes("/opt/axon/libaxon_pjrt.so")
            _m = types.ModuleType("antenv.axon_hooks")
            _m.get_axon_ntff_profile_hook = lambda: _hook
            _m.set_axon_ntff_profile_hook = lambda h: None
            sys.modules["antenv.axon_hooks"] = _m
        except Exception:
            pass


def _build():
    nc = bacc.Bacc(None, target_bir_lowering=False, debug=False, num_devices=M)

    # ---- per-core inputs (host pre-transformed)
    xT_d = nc.dram_tensor("xT", [D_IN, R], f32, kind="ExternalInput")
    Lres_d = nc.dram_tensor("Lres", [NRES, 2, 128, 4096], fp8, kind="ExternalInput")
    Lstr_d = nc.dram_tensor("Lstr", [2 * NSTR, 128, 4096], fp8, kind="ExternalInput")
    embWt_d = nc.dram_tensor("embWt", [D_IN, TD], f32, kind="ExternalInput")
    embB_d = nc.dram_tensor("embB", [128, 1], f32, kind="ExternalInput")
    w1_d = nc.dram_tensor("w1", [NL, 128, TD], f32, kind="ExternalInput")
    b1_d = nc.dram_tensor("b1", [NL, 128, 1], f32, kind="ExternalInput")
    w2_d = nc.dram_tensor("w2", [NL, 128, TD], f32, kind="ExternalInput")
    b2s_d = nc.dram_tensor("b2s", [NL, 128, 1], f32, kind="ExternalInput")
    b2c_d = nc.dram_tensor("b2c", [NL, 128, 1], f32, kind="ExternalInput")
    ltw_d = nc.dram_tensor("ltw", [NL, 128, TD], f32, kind="ExternalInput")
    ltb_d = nc.dram_tensor("ltb", [NL, 128, 1], f32, kind="ExternalInput")
    outw_d = nc.dram_tensor("outw", [128, D_OUT], f32, kind="ExternalInput")
    outb_d = nc.dram_tensor("outb", [D_OUT, 1], f32, kind="ExternalInput")

    outT_d = nc.dram_tensor("outT", [D_OUT, R], f32, kind="ExternalOutput")

    # collective buffers: per phase (A: chunks 0+2, B: chunks 1+3), parity
    loc_d = [
        [nc.dram_tensor(f"loc{p}_{q}", [128, 512], bf16) for q in range(2)]
        for p in range(2)
    ]
    full_d = [
        [
            nc.dram_tensor(f"full{p}_{q}", [M, 128, 512], bf16, addr_space="Shared")
            for q in range(2)
        ]
        for p in range(2)
    ]
    RG = [list(range(M))]

    with tile.TileContext(nc) as tc:
        with (
            tc.tile_pool(name="resp", bufs=1) as resp,
            tc.tile_pool(name="lpool", bufs=8) as lpool,
            tc.tile_pool(name="ttp", bufs=20) as ttp,
            tc.tile_pool(name="packp", bufs=2) as packp,
            tc.tile_pool(name="acc", bufs=2, space="PSUM") as accp,
            tc.tile_pool(name="phip", bufs=2, space="PSUM") as phip,
            tc.tile_pool(name="trp", bufs=2, space="PSUM") as trp,
            tc.tile_pool(name="wk1", bufs=1) as wk1,
            tc.tile_pool(name="wk2", bufs=2) as wk2,
            tc.tile_pool(name="sg", bufs=1) as sg,
        ):
            # ---- persistent SBUF state
            ident2 = sg.tile([128, TD], bf16)
            make_identity(nc, ident2[0:TD, :])
            make_identity(nc, ident2[TD:128, :])
            h_sb = sg.tile([128, R // 2], f32)
            res_sb = sg.tile([128, R // 2], f32)
            c2_sb = sg.tile([128, R // 2], f32)
            ssgn_sb = sg.tile([128, R // 2], f32)

            # weights resident in SBUF
            embWt = sg.tile([D_IN, TD], f32)
            nc.sync.dma_start(out=embWt[:], in_=embWt_d[:, :])
            embB = sg.tile([128, 1], f32)
            nc.sync.dma_start(out=embB[:], in_=embB_d[:, :])
            w1 = [sg.tile([128, TD], f32, tag=f"w1_{i}", name=f"w1_{i}") for i in range(NL)]
            b1 = [sg.tile([128, 1], f32, tag=f"b1_{i}", name=f"b1_{i}") for i in range(NL)]
            w2 = [sg.tile([128, TD], f32, tag=f"w2_{i}", name=f"w2_{i}") for i in range(NL)]
            b2s = [sg.tile([128, 1], f32, tag=f"b2s_{i}", name=f"b2s_{i}") for i in range(NL)]
            b2c = [sg.tile([128, 1], f32, tag=f"b2c_{i}", name=f"b2c_{i}") for i in range(NL)]
            ltw = [sg.tile([128, TD], f32, tag=f"ltw_{i}", name=f"ltw_{i}") for i in range(NL)]
            ltb = [sg.tile([128, 1], f32, tag=f"ltb_{i}", name=f"ltb_{i}") for i in range(NL)]
            for i in range(NL):
                nc.sync.dma_start(out=w1[i][:], in_=w1_d[i, :, :])
                nc.sync.dma_start(out=b1[i][:], in_=b1_d[i, :, :])
                nc.sync.dma_start(out=w2[i][:], in_=w2_d[i, :, :])
                nc.sync.dma_start(out=b2s[i][:], in_=b2s_d[i, :, :])
                nc.sync.dma_start(out=b2c[i][:], in_=b2c_d[i, :, :])
                nc.sync.dma_start(out=ltw[i][:], in_=ltw_d[i, :, :])
                nc.sync.dma_start(out=ltb[i][:], in_=ltb_d[i, :, :])
            outw = sg.tile([128, D_OUT], f32)
            nc.sync.dma_start(out=outw[:], in_=outw_d[:, :])
            outb = sg.tile([D_OUT, 1], f32)
            nc.sync.dma_start(out=outb[:], in_=outb_d[:, :])

            # resident L units (both phases)
            lres = []
            for t in range(NRES):
                pair = []
                for p in range(2):
                    lt = sg.tile([128, 4096], fp8, tag=f"lr{t}_{p}", name=f"lr{t}_{p}")
                    nc.sync.dma_start(out=lt[:], in_=Lres_d[t, p, :, :])
                    pair.append(lt)
                lres.append(pair)

            # ---- embedding: h = emb(x)  (folded [128, 1024] layout)
            xt1 = lpool.tile([D_IN, R // 2], f32, tag="lt", name="xt1")
            nc.sync.dma_start(out=xt1[:], in_=xT_d[:, 0 : R // 2])
            xt2 = lpool.tile([D_IN, R // 2], f32, tag="lt", name="xt2")
            nc.sync.dma_start(out=xt2[:], in_=xT_d[:, R // 2 : R])
            hps = phip.tile([128, 1024], f32, tag="phi")
            for n in range(2):
                sl = slice(n * 512, (n + 1) * 512)
                nc.tensor.matmul(
                    hps[0:TD, sl], embWt[:], xt1[:, sl],
                    start=True, stop=True, tile_position=(0, 0),
                )
                nc.tensor.matmul(
                    hps[TD:128, sl], embWt[:], xt2[:, sl],
                    start=True, stop=True, tile_position=(0, 64),
                )
            nc.vector.tensor_scalar_add(h_sb[:], hps[:], embB[:])

            def emit_pack_phase(src_sb, scale, ph, par, tag):
                """Cast src[:, ph*512:+512]*scale -> bf16, transpose to
                node-major pack [128, 512], DMA to loc, AllGather."""
                sl = slice(ph * 512, (ph + 1) * 512)
                tbf = wk2.tile([128, 512], bf16, tag="tbf", name=f"tbf_{tag}")
                nc.scalar.activation(
                    tbf[:], src_sb[:, sl],
                    mybir.ActivationFunctionType.Copy, scale=scale,
                )
                pack = packp.tile([128, 512], bf16, tag=f"pk{ph}", name=f"pk_{tag}")
                for half in range(2):
                    t_ps = trp.tile([128, 256], bf16, tag="trp")
                    for b in range(4):
                        nc.tensor.transpose(
                            t_ps[:, b * TD : (b + 1) * TD],
                            tbf[half * TD : (half + 1) * TD, b * 128 : (b + 1) * 128],
                            ident2[half * TD : (half + 1) * TD, :],
                        )
                    nc.vector.tensor_copy(
                        pack[:, half * 256 : (half + 1) * 256], t_ps[:]
                    )
                nc.gpsimd.dma_start(out=loc_d[ph][par][:, :], in_=pack[:])
                nc.gpsimd.collective_compute(
                    "AllGather",
                    mybir.AluOpType.bypass,
                    replica_groups=RG,
                    ins=[loc_d[ph][par][:, :]],
                    outs=[full_d[ph][par][:, :, :]],
                )
                return pack

            def fetch_slots(par, tag):
                """Static per-slot reads of both phase AG buffers (16 DMAs)."""
                tts = {}
                for ph in range(2):
                    for j in range(M):
                        tt = ttp.tile(
                            [128, 512], bf16, tag="tt", name=f"tt_{tag}_{ph}_{j}"
                        )
                        nc.gpsimd.dma_start(
                            out=tt[:], in_=full_d[ph][par][j, :, :]
                        )
                        tts[(ph, j)] = tt
                return tts

            def emit_group(accs, ph, units, tts, stiles):
                """Matmuls for one (phase, unit-group)."""
                acc = accs[ph]
                for t in units:
                    c, j = ORDER[t]
                    cph = 0 if c in (0, 2) else 1
                    coff = 256 if c >= 2 else 0
                    lh_tile = tts[(cph, j)]
                    if t < NRES:
                        rhs = lres[t][ph]
                    else:
                        rhs = stiles[(ph, t)]
                    for jj in range(4):
                        lh = lh_tile[:, coff + jj * TD : coff + (jj + 1) * TD]
                        first = t == 0 and jj == 0
                        last = t == 31 and jj == 3
                        nc.tensor.matmul(
                            acc[0:TD, :],
                            lh,
                            rhs[:, jj * 1024 : jj * 1024 + 512],
                            start=first, stop=last, tile_position=(0, 0),
                        )
                        nc.tensor.matmul(
                            acc[TD:128, :],
                            lh,
                            rhs[:, jj * 1024 + 512 : (jj + 1) * 1024],
                            start=first, stop=last, tile_position=(0, 64),
                        )

            for i in range(NL):
                # ---- phi MLP -> signed duplicated angles -> sin/cos
                ps1 = phip.tile([128, 1024], f32, tag="phi", name=f"ps1_{i}")
                for n in range(2):
                    sl = slice(n * 512, (n + 1) * 512)
                    nc.tensor.matmul(
                        ps1[0:TD, sl], w1[i][0:TD, :], h_sb[0:TD, sl],
                        start=True, stop=True, tile_position=(0, 0),
                    )
                    nc.tensor.matmul(
                        ps1[TD:128, sl], w1[i][TD:128, :], h_sb[TD:128, sl],
                        start=True, stop=True, tile_position=(64, 64),
                    )
                g_sb = wk1.tile([128, 1024], f32, tag="g", name=f"g_{i}")
                nc.scalar.activation(
                    g_sb[:], ps1[:], mybir.ActivationFunctionType.Gelu, bias=b1[i][:]
                )
                ps2 = phip.tile([128, 1024], f32, tag="phi", name=f"ps2_{i}")
                for n in range(2):
                    sl = slice(n * 512, (n + 1) * 512)
                    nc.tensor.matmul(
                        ps2[0:TD, sl], w2[i][0:TD, :], g_sb[0:TD, sl],
                        start=True, stop=True, tile_position=(0, 0),
                    )
                    nc.tensor.matmul(
                        ps2[TD:128, sl], w2[i][TD:128, :], g_sb[TD:128, sl],
                        start=True, stop=True, tile_position=(64, 64),
                    )
                nc.scalar.activation(
                    ssgn_sb[:], ps2[:], mybir.ActivationFunctionType.Sin, bias=b2s[i][:]
                )
                nc.scalar.activation(
                    c2_sb[:], ps2[:], mybir.ActivationFunctionType.Sin, bias=b2c[i][:]
                )

                # ---- rotate into bundle frame (folded layout)
                swap = wk1.tile([128, 1024], f32, tag="swap", name=f"swap_{i}")
                nc.vector.tensor_copy(swap[0:B, :], h_sb[B:TD, :])
                nc.vector.tensor_copy(swap[B:TD, :], h_sb[0:B, :])
                nc.vector.tensor_copy(swap[TD : TD + B, :], h_sb[TD + B : 128, :])
                nc.vector.tensor_copy(swap[TD + B : 128, :], h_sb[TD : TD + B, :])
                rot = wk1.tile([128, 1024], f32, tag="rot", name=f"rot_{i}")
                nc.vector.tensor_mul(rot[:], c2_sb[:], h_sb[:])
                tmp = wk1.tile([128, 1024], f32, tag="tmp", name=f"tmp_{i}")
                nc.vector.tensor_mul(tmp[:], ssgn_sb[:], swap[:])
                nc.vector.tensor_add(rot[:], rot[:], tmp[:])

                # ---- linear transform H = lt(rot); res = H
                psH = phip.tile([128, 1024], f32, tag="phi", name=f"psH_{i}")
                for n in range(2):
                    sl = slice(n * 512, (n + 1) * 512)
                    nc.tensor.matmul(
                        psH[0:TD, sl], ltw[i][0:TD, :], rot[0:TD, sl],
                        start=True, stop=True, tile_position=(0, 0),
                    )
                    nc.tensor.matmul(
                        psH[TD:128, sl], ltw[i][TD:128, :], rot[TD:128, sl],
                        start=True, stop=True, tile_position=(64, 64),
                    )
                nc.vector.tensor_scalar_add(res_sb[:], psH[:], ltb[i][:])

                # term_0 packs (consumed by step 1); scale = -1/1
                emit_pack_phase(res_sb, -1.0, 0, 0, f"h{i}")
                emit_pack_phase(res_sb, -1.0, 1, 0, f"h{i}b")
                # slot fetches for step 1 (queued behind the H AllGathers)
                tts = fetch_slots(0, f"h{i}")

                # ---- Taylor diffusion
                for k in range(1, K + 1):
                    pc = (k - 1) % 2   # parity consumed
                    pp = k % 2         # parity produced

                    def drain_phase(accs, ph, k=k, i=i):
                        """Pack+AG first (critical path), then res += acc*DSC."""
                        acc = accs[ph]
                        sl = slice(ph * 512, (ph + 1) * 512)
                        pack = None
                        if k < K:
                            tbf = wk2.tile(
                                [128, 512], bf16, tag="tbf", name=f"tb_{i}_{k}_{ph}"
                            )
                            nc.scalar.activation(
                                tbf[:], acc[:],
                                mybir.ActivationFunctionType.Copy,
                                scale=-DSC / (k + 1),
                            )
                            pack = packp.tile(
                                [128, 512], bf16, tag=f"pk{ph}", name=f"pk_{i}_{k}_{ph}"
                            )
                            for half in range(2):
                                t_ps = trp.tile([128, 256], bf16, tag="trp")
                                for b in range(4):
                                    nc.tensor.transpose(
                                        t_ps[:, b * TD : (b + 1) * TD],
                                        tbf[half * TD : (half + 1) * TD,
                                            b * 128 : (b + 1) * 128],
                                        ident2[half * TD : (half + 1) * TD, :],
                                    )
                                nc.vector.tensor_copy(
                                    pack[:, half * 256 : (half + 1) * 256], t_ps[:]
                                )
                            nc.gpsimd.dma_start(out=loc_d[ph][pp][:, :], in_=pack[:])
                            nc.gpsimd.collective_compute(
                                "AllGather",
                                mybir.AluOpType.bypass,
                                replica_groups=RG,
                                ins=[loc_d[ph][pp][:, :]],
                                outs=[full_d[ph][pp][:, :, :]],
                            )
                        stage = wk2.tile(
                            [128, 512], f32, tag="stage", name=f"st_{i}_{k}_{ph}"
                        )
                        nc.scalar.activation(
                            stage[:], acc[:],
                            mybir.ActivationFunctionType.Copy, scale=DSC,
                        )
                        nc.vector.tensor_add(res_sb[:, sl], res_sb[:, sl], stage[:])
                        return pack

                    # streamed L half-units, in exact consumption order
                    stiles = {}
                    for s, (ph, t) in enumerate(STREAM_ORDER):
                        lt = lpool.tile(
                            [128, 4096], fp8, tag="lt", name=f"ls_{i}_{k}_{s}"
                        )
                        nc.sync.dma_start(out=lt[:], in_=Lstr_d[s, :, :])
                        stiles[(ph, t)] = lt
                    accs = [
                        accp.tile([128, 512], f32, tag="acc", name=f"acc_{i}_{k}_0"),
                        accp.tile([128, 512], f32, tag="acc", name=f"acc_{i}_{k}_1"),
                    ]
                    # AG_B-independent work first (both phases)
                    for ph, grp in GROUPS_PRE:
                        emit_group(accs, ph, grp, tts, stiles)
                    # phase A tail -> drain+AG_A early
                    for ph, grp in GROUPS_A_TAIL:
                        emit_group(accs, ph, grp, tts, stiles)
                    drain_phase(accs, 0)
                    # phase B tail -> drain+AG_B
                    for ph, grp in GROUPS_B_TAIL:
                        emit_group(accs, ph, grp, tts, stiles)
                    drain_phase(accs, 1)
                    if k < K:
                        # slot fetches for step k+1: queued on gpsimd behind
                        # this step's AllGathers; they wait on the AG sems,
                        # then all fire across DMA queues in parallel
                        tts = fetch_slots(pp, f"{i}_{k}")

                # ---- rotate back, gelu, residual
                swap2 = wk1.tile([128, 1024], f32, tag="swap", name=f"swap2_{i}")
                nc.vector.tensor_copy(swap2[0:B, :], res_sb[B:TD, :])
                nc.vector.tensor_copy(swap2[B:TD, :], res_sb[0:B, :])
                nc.vector.tensor_copy(swap2[TD : TD + B, :], res_sb[TD + B : 128, :])
                nc.vector.tensor_copy(swap2[TD + B : 128, :], res_sb[TD : TD + B, :])
                rot2 = wk1.tile([128, 1024], f32, tag="rot", name=f"rot2_{i}")
                nc.vector.tensor_mul(rot2[:], c2_sb[:], res_sb[:])
                tmp2 = wk1.tile([128, 1024], f32, tag="tmp", name=f"tmp2_{i}")
                nc.vector.tensor_mul(tmp2[:], ssgn_sb[:], swap2[:])
                nc.vector.tensor_sub(rot2[:], rot2[:], tmp2[:])
                g2 = wk1.tile([128, 1024], f32, tag="g", name=f"g2_{i}")
                nc.scalar.activation(
                    g2[:], rot2[:], mybir.ActivationFunctionType.Gelu
                )
                nc.vector.tensor_add(h_sb[:], h_sb[:], g2[:])

            # ---- output projection (per node-half)
            for half in range(2):
                pso = phip.tile([D_OUT, 1024], f32, tag="phi", name=f"pso_{half}")
                hsl = slice(half * TD, (half + 1) * TD)
                for n in range(2):
                    sl = slice(n * 512, (n + 1) * 512)
                    nc.tensor.matmul(
                        pso[:, sl], outw[hsl, :], h_sb[hsl, sl],
                        start=True, stop=True,
                        tile_position=(64 * half, 0),
                    )
                o_sb = wk1.tile([D_OUT, 1024], f32, tag="g", name=f"o_{half}")
                nc.vector.tensor_scalar_add(o_sb[:], pso[:], outb[:])
                nc.sync.dma_start(
                    out=outT_d[:, half * 1024 : (half + 1) * 1024], in_=o_sb[:]
                )

    nc.compile()
    return nc


def kernel(**inputs):
    x = np.asarray(inputs["x"], dtype=np.float32)
    L = np.asarray(inputs["L"], dtype=np.float32)
    emb_W = np.asarray(inputs["emb_W"], dtype=np.float32)
    emb_b = np.asarray(inputs["emb_b"], dtype=np.float32)
    phi_W1 = np.asarray(inputs["phi_W1"], dtype=np.float32)
    phi_b1 = np.asarray(inputs["phi_b1"], dtype=np.float32)
    phi_W2 = np.asarray(inputs["phi_W2"], dtype=np.float32)
    phi_b2 = np.asarray(inputs["phi_b2"], dtype=np.float32)
    lt_W = np.asarray(inputs["lt_W"], dtype=np.float32)
    lt_b = np.asarray(inputs["lt_b"], dtype=np.float32)
    out_W = np.asarray(inputs["out_W"], dtype=np.float32)
    out_b = np.asarray(inputs["out_b"], dtype=np.float32)

    perm = np.concatenate([np.arange(0, TD, 2), np.arange(1, TD, 2)])

    def stack2(a):  # duplicate along partition axis
        return np.ascontiguousarray(np.concatenate([a, a], axis=0))

    embWt = np.ascontiguousarray(emb_W.T[:, perm])
    embB = stack2(emb_b[perm][:, None])
    w1 = np.ascontiguousarray(
        np.stack([stack2(phi_W1[i].T[perm, :]) for i in range(NL)])
    )
    b1 = np.ascontiguousarray(
        np.stack([stack2(phi_b1[i][:, None]) for i in range(NL)])
    )
    w2 = np.ascontiguousarray(
        np.stack(
            [
                stack2(np.concatenate([-phi_W2[i].T, phi_W2[i].T], axis=1))
                for i in range(NL)
            ]
        )
    )
    b2s = np.ascontiguousarray(
        np.stack(
            [
                stack2(np.concatenate([-phi_b2[i], phi_b2[i]])[:, None])
                for i in range(NL)
            ]
        )
    )
    b2c = (b2s + np.float32(np.pi / 2)).astype(np.float32)
    ltw = np.ascontiguousarray(
        np.stack([stack2(lt_W[i].T[perm][:, perm]) for i in range(NL)])
    )
    ltb = np.ascontiguousarray(
        np.stack([stack2(lt_b[i][perm][:, None]) for i in range(NL)])
    )
    outw = stack2(np.ascontiguousarray(out_W.T[perm, :]))
    outb = np.ascontiguousarray(out_b[:, None])

    # quantize L once: transposed, scaled, e3m4
    L8T = np.ascontiguousarray(L.T * np.float32(LSC)).astype(F8)

    shared = {
        "embWt": embWt, "embB": embB, "w1": w1, "b1": b1, "w2": w2,
        "b2s": b2s, "b2c": b2c, "ltw": ltw, "ltb": ltb,
        "outw": outw, "outb": outb,
    }

    # column selections per phase: (cg0 block | cg1 block)
    colsel = [np.r_[0:512, 1024:1536], np.r_[512:1024, 1536:2048]]

    in_maps = []
    for core in range(M):
        # row order: per unit t = (chunk, rank offset), 512 global rows
        row_order = np.concatenate(
            [j * R + c * 512 + np.arange(512) for (c, j) in ORDER]
        )
        Y = L8T[row_order][:, core * R : (core + 1) * R]  # [16384, 2048]
        Y4 = Y.reshape(32, 4, 128, R)  # (t, jj, p, n)
        tiles = np.empty((32, 2, 128, 4096), dtype=F8)
        for p in range(2):
            Z = Y4[:, :, :, colsel[p]]  # [32, 4, 128, 1024]
            tiles[:, p] = np.transpose(Z, (0, 2, 1, 3)).reshape(32, 128, 4096)
        Lres = np.ascontiguousarray(tiles[:NRES])
        Lstr = np.ascontiguousarray(
            np.stack([tiles[t, ph] for (ph, t) in STREAM_ORDER])
        )
        in_maps.append(
            {
                "xT": np.ascontiguousarray(x[core * R : (core + 1) * R].T),
                "Lres": Lres,
                "Lstr": Lstr,
                **shared,
            }
        )

    if "nc" not in _CACHE:
        _CACHE["nc"] = _build()
    nc = _CACHE["nc"]

    trace = bool(os.environ.get("BUNN_TRACE"))
    if trace:
        _install_ntff_shim()
    res = run_bass_kernel_spmd(nc, in_maps, list(range(M)), trace=trace)
    if trace and res.exec_time_ns is not None:
        print(f"HW exec time: {res.exec_time_ns} ns")
        _CACHE["exec_time_ns"] = res.exec_time_ns

    out = np.empty((N, D_OUT), dtype=np.float32)
    for core in range(M):
        out[core * R : (core + 1) * R, :] = res.results[core]["outT"].T
    return out


# revision 15
# speedup vs baseline: 1.0236x; 1.0236x over previous
"""Trainium2 Bass kernel for BuNN (nn_BuNN_10797547782311).

Row-shard L over 8 cores. Per Taylor step, each core contracts its
[2048, 16384] L block (fp8 e3m4, x128 scale, pre-transposed/pre-tiled on
host) against the replicated bf16 term. Key optimizations over v1:

- L stored as fp8 e3m4 (x128): halves HBM traffic; PE-exact (probed).
- 14 of 32 contraction units resident in SBUF (~112KB/partition):
  per-step streamed DMA drops to 18MB.
- Column-tiled matmul pairs (tile_position (0,0)/(0,64)): two 64-wide
  matmuls run concurrently on both halves of the PE array.
- Two-phase steps: output node-blocks {0,2} complete first, are cast /
  transposed / AllGathered while blocks {1,3} compute -> 2 AGs per step
  whose ~24us latency hides under the opposite phase's matmuls.
- Work tiles folded to [128, 1024]: partitions = td x node-half.

Node-parallel math identical to v1 (td-permuted rotation layout, signed
duplicated angles with sin(x + pi/2) for cos).
"""

import os
import sys
import types

import numpy as np
import ml_dtypes

import concourse.bacc as bacc
import concourse.tile as tile
from concourse import mybir
from concourse.bass_utils import run_bass_kernel_spmd
from concourse.bass import ds
from concourse.masks import make_identity

# Problem config (hardcoded)
N, D_IN, D_OUT = 16384, 128, 40
B = 32
TD = 2 * B          # 64
NL = 4              # layers
K = 8               # Taylor steps
M = 8               # cores
R = N // M          # 2048 rows per core

NRES = 13           # resident contraction units (of 32)
NSTR = 32 - NRES    # streamed units
LSC = 128.0         # L fp8 scale (power of 2)
DSC = 1.0 / LSC     # descale for psum -> term

f32 = mybir.dt.float32
bf16 = mybir.dt.bfloat16
fp8 = mybir.dt.float8e3
BF = ml_dtypes.bfloat16
F8 = ml_dtypes.float8_e3m4

# contraction-unit consumption order: (chunk, absolute rank slot j).
# All 8 slots (self included) are consumed from the AllGather output at
# static offsets -- no dynamic-offset DMAs anywhere.
ORDER = [(c, j) for c in (0, 2, 1, 3) for j in range(M)]

# unit-index groups (into ORDER): c0=0..7, c2=8..15, c1=16..23, c3=24..31
GC0 = list(range(0, 8))
GC2 = list(range(8, 16))
GC1 = list(range(16, 24))
GC3 = list(range(24, 32))
# per-step global emission order: ALL of phase A first so AG_A (the
# critical-cycle collective) triggers as early as possible, then phase B.
GROUPS_PRE = [(0, GC0), (0, GC2)]
GROUPS_A_TAIL = [(0, GC1), (0, GC3)]
GROUPS_B_TAIL = [(1, GC0), (1, GC2), (1, GC1), (1, GC3)]
STREAM_ORDER = [
    (ph, t)
    for (ph, grp) in GROUPS_PRE + GROUPS_A_TAIL + GROUPS_B_TAIL
    for t in grp
    if t >= NRES
]

_CACHE = {}


def _install_ntff_shim():
    try:
        from antenv.axon_hooks import get_axon_ntff_profile_hook  # noqa: F401
    except ImportError:
        try:
            from trn_agent_boot.trn_boot import _ntff_profile_via_ctypes

            _hook = _ntff_profile_via_ctypes("/opt/axon/libaxon_pjrt.so")
            _m = types.ModuleType("antenv.axon_hooks")
            _m.get_axon_ntff_profile_hook = lambda: _hook
            _m.set_axon_ntff_profile_hook = lambda h: None
            sys.modules["antenv.axon_hooks"] = _m
        except Exception:
            pass


def _build():
    nc = bacc.Bacc(None, target_bir_lowering=False, debug=False, num_devices=M)

    # ---- per-core inputs (host pre-transformed)
    xT_d = nc.dram_tensor("xT", [D_IN, R], f32, kind="ExternalInput")
    Lres_d = nc.dram_tensor("Lres", [NRES, 2, 128, 4096], fp8, kind="ExternalInput")
    Lstr_d = nc.dram_tensor("Lstr", [2 * NSTR, 128, 4096], fp8, kind="ExternalInput")
    embWt_d = nc.dram_tensor("embWt", [D_IN, TD], f32, kind="ExternalInput")
    embB_d = nc.dram_tensor("embB", [128, 1], f32, kind="ExternalInput")
    w1_d = nc.dram_tensor("w1", [NL, 128, TD], f32, kind="ExternalInput")
    b1_d = nc.dram_tensor("b1", [NL, 128, 1], f32, kind="ExternalInput")
    w2_d = nc.dram_tensor("w2", [NL, 128, TD], f32, kind="ExternalInput")
    b2s_d = nc.dram_tensor("b2s", [NL, 128, 1], f32, kind="ExternalInput")
    b2c_d = nc.dram_tensor("b2c", [NL, 128, 1], f32, kind="ExternalInput")
    ltw_d = nc.dram_tensor("ltw", [NL, 128, TD], f32, kind="ExternalInput")
    ltb_d = nc.dram_tensor("ltb", [NL, 128, 1], f32, kind="ExternalInput")
    outw_d = nc.dram_tensor("outw", [128, D_OUT], f32, kind="ExternalInput")
    outb_d = nc.dram_tensor("outb", [D_OUT, 1], f32, kind="ExternalInput")

    outT_d = nc.dram_tensor("outT", [D_OUT, R], f32, kind="ExternalOutput")

    # collective buffers: per phase (A: chunks 0+2, B: chunks 1+3), parity
    loc_d = [
        [nc.dram_tensor(f"loc{p}_{q}", [128, 512], bf16) for q in range(2)]
        for p in range(2)
    ]
    full_d = [
        [
            nc.dram_tensor(f"full{p}_{q}", [M, 128, 512], bf16, addr_space="Shared")
            for q in range(2)
        ]
        for p in range(2)
    ]
    RG = [list(range(M))]

    with tile.TileContext(nc) as tc:
        with (
            tc.tile_pool(name="resp", bufs=1) as resp,
            tc.tile_pool(name="lpool", bufs=8) as lpool,
            tc.tile_pool(name="ttp", bufs=20) as ttp,
            tc.tile_pool(name="packp", bufs=2) as packp,
            tc.tile_pool(name="acc", bufs=2, space="PSUM") as accp,
            tc.tile_pool(name="phip", bufs=2, space="PSUM") as phip,
            tc.tile_pool(name="trp", bufs=2, space="PSUM") as trp,
            tc.tile_pool(name="wk1", bufs=1) as wk1,
            tc.tile_pool(name="wk2", bufs=2) as wk2,
            tc.tile_pool(name="sg", bufs=1) as sg,
        ):
            # ---- persistent SBUF state
            ident2 = sg.tile([128, TD], bf16)
            make_identity(nc, ident2[0:TD, :])
            make_identity(nc, ident2[TD:128, :])
            h_sb = sg.tile([128, R // 2], f32)
            res_sb = sg.tile([128, R // 2], f32)
            c2_sb = sg.tile([128, R // 2], f32)
            ssgn_sb = sg.tile([128, R // 2], f32)

            # weights resident in SBUF
            embWt = sg.tile([D_IN, TD], f32)
            nc.sync.dma_start(out=embWt[:], in_=embWt_d[:, :])
            embB = sg.tile([128, 1], f32)
            nc.sync.dma_start(out=embB[:], in_=embB_d[:, :])
            w1 = [sg.tile([128, TD], f32, tag=f"w1_{i}", name=f"w1_{i}") for i in range(NL)]
            b1 = [sg.tile([128, 1], f32, tag=f"b1_{i}", name=f"b1_{i}") for i in range(NL)]
            w2 = [sg.tile([128, TD], f32, tag=f"w2_{i}", name=f"w2_{i}") for i in range(NL)]
            b2s = [sg.tile([128, 1], f32, tag=f"b2s_{i}", name=f"b2s_{i}") for i in range(NL)]
            b2c = [sg.tile([128, 1], f32, tag=f"b2c_{i}", name=f"b2c_{i}") for i in range(NL)]
            ltw = [sg.tile([128, TD], f32, tag=f"ltw_{i}", name=f"ltw_{i}") for i in range(NL)]
            ltb = [sg.tile([128, 1], f32, tag=f"ltb_{i}", name=f"ltb_{i}") for i in range(NL)]
            for i in range(NL):
                nc.sync.dma_start(out=w1[i][:], in_=w1_d[i, :, :])
                nc.sync.dma_start(out=b1[i][:], in_=b1_d[i, :, :])
                nc.sync.dma_start(out=w2[i][:], in_=w2_d[i, :, :])
                nc.sync.dma_start(out=b2s[i][:], in_=b2s_d[i, :, :])
                nc.sync.dma_start(out=b2c[i][:], in_=b2c_d[i, :, :])
                nc.sync.dma_start(out=ltw[i][:], in_=ltw_d[i, :, :])
                nc.sync.dma_start(out=ltb[i][:], in_=ltb_d[i, :, :])
            outw = sg.tile([128, D_OUT], f32)
            nc.sync.dma_start(out=outw[:], in_=outw_d[:, :])
            outb = sg.tile([D_OUT, 1], f32)
            nc.sync.dma_start(out=outb[:], in_=outb_d[:, :])

            # resident L units (both phases)
            lres = []
            for t in range(NRES):
                pair = []
                for p in range(2):
                    lt = sg.tile([128, 4096], fp8, tag=f"lr{t}_{p}", name=f"lr{t}_{p}")
                    nc.sync.dma_start(out=lt[:], in_=Lres_d[t, p, :, :])
                    pair.append(lt)
                lres.append(pair)

            # ---- embedding: h = emb(x)  (folded [128, 1024] layout)
            xt1 = lpool.tile([D_IN, R // 2], f32, tag="lt", name="xt1")
            nc.sync.dma_start(out=xt1[:], in_=xT_d[:, 0 : R // 2])
            xt2 = lpool.tile([D_IN, R // 2], f32, tag="lt", name="xt2")
            nc.sync.dma_start(out=xt2[:], in_=xT_d[:, R // 2 : R])
            hps = phip.tile([128, 1024], f32, tag="phi")
            for n in range(2):
                sl = slice(n * 512, (n + 1) * 512)
                nc.tensor.matmul(
                    hps[0:TD, sl], embWt[:], xt1[:, sl],
                    start=True, stop=True, tile_position=(0, 0),
                )
                nc.tensor.matmul(
                    hps[TD:128, sl], embWt[:], xt2[:, sl],
                    start=True, stop=True, tile_position=(0, 64),
                )
            nc.vector.tensor_scalar_add(h_sb[:], hps[:], embB[:])

            def emit_pack_phase(src_sb, scale, ph, par, tag):
                """Cast src[:, ph*512:+512]*scale -> bf16, transpose to
                node-major pack [128, 512], DMA to loc, AllGather."""
                sl = slice(ph * 512, (ph + 1) * 512)
                tbf = wk2.tile([128, 512], bf16, tag="tbf", name=f"tbf_{tag}")
                nc.scalar.activation(
                    tbf[:], src_sb[:, sl],
                    mybir.ActivationFunctionType.Copy, scale=scale,
                )
                pack = packp.tile([128, 512], bf16, tag=f"pk{ph}", name=f"pk_{tag}")
                for half in range(2):
                    t_ps = trp.tile([128, 256], bf16, tag="trp")
                    for b in range(4):
                        nc.tensor.transpose(
                            t_ps[:, b * TD : (b + 1) * TD],
                            tbf[half * TD : (half + 1) * TD, b * 128 : (b + 1) * 128],
                            ident2[half * TD : (half + 1) * TD, :],
                        )
                    nc.vector.tensor_copy(
                        pack[:, half * 256 : (half + 1) * 256], t_ps[:]
                    )
                nc.gpsimd.dma_start(out=loc_d[ph][par][:, :], in_=pack[:])
                nc.gpsimd.collective_compute(
                    "AllGather",
                    mybir.AluOpType.bypass,
                    replica_groups=RG,
                    ins=[loc_d[ph][par][:, :]],
                    outs=[full_d[ph][par][:, :, :]],
                )
                return pack

            def fetch_slots(par, tag):
                """Static per-slot reads of both phase AG buffers (16 DMAs)."""
                tts = {}
                for ph in range(2):
                    for j in range(M):
                        tt = ttp.tile(
                            [128, 512], bf16, tag="tt", name=f"tt_{tag}_{ph}_{j}"
                        )
                        nc.gpsimd.dma_start(
                            out=tt[:], in_=full_d[ph][par][j, :, :]
                        )
                        tts[(ph, j)] = tt
                return tts

            def emit_group(accs, ph, units, tts, stiles):
                """Matmuls for one (phase, unit-group)."""
                acc = accs[ph]
                for t in units:
                    c, j = ORDER[t]
                    cph = 0 if c in (0, 2) else 1
                    coff = 256 if c >= 2 else 0
                    lh_tile = tts[(cph, j)]
                    if t < NRES:
                        rhs = lres[t][ph]
                    else:
                        rhs = stiles[(ph, t)]
                    for jj in range(4):
                        lh = lh_tile[:, coff + jj * TD : coff + (jj + 1) * TD]
                        first = t == 0 and jj == 0
                        last = t == 31 and jj == 3
                        nc.tensor.matmul(
                            acc[0:TD, :],
                            lh,
                            rhs[:, jj * 1024 : jj * 1024 + 512],
                            start=first, stop=last, tile_position=(0, 0),
                        )
                        nc.tensor.matmul(
                            acc[TD:128, :],
                            lh,
                            rhs[:, jj * 1024 + 512 : (jj + 1) * 1024],
                            start=first, stop=last, tile_position=(0, 64),
                        )

            for i in range(NL):
                # ---- phi MLP -> signed duplicated angles -> sin/cos
                ps1 = phip.tile([128, 1024], f32, tag="phi", name=f"ps1_{i}")
                for n in range(2):
                    sl = slice(n * 512, (n + 1) * 512)
                    nc.tensor.matmul(
                        ps1[0:TD, sl], w1[i][0:TD, :], h_sb[0:TD, sl],
                        start=True, stop=True, tile_position=(0, 0),
                    )
                    nc.tensor.matmul(
                        ps1[TD:128, sl], w1[i][TD:128, :], h_sb[TD:128, sl],
                        start=True, stop=True, tile_position=(64, 64),
                    )
                g_sb = wk1.tile([128, 1024], f32, tag="g", name=f"g_{i}")
                nc.scalar.activation(
                    g_sb[:], ps1[:], mybir.ActivationFunctionType.Gelu, bias=b1[i][:]
                )
                ps2 = phip.tile([128, 1024], f32, tag="phi", name=f"ps2_{i}")
                for n in range(2):
                    sl = slice(n * 512, (n + 1) * 512)
                    nc.tensor.matmul(
                        ps2[0:TD, sl], w2[i][0:TD, :], g_sb[0:TD, sl],
                        start=True, stop=True, tile_position=(0, 0),
                    )
                    nc.tensor.matmul(
                        ps2[TD:128, sl], w2[i][TD:128, :], g_sb[TD:128, sl],
                        start=True, stop=True, tile_position=(64, 64),
                    )
                nc.scalar.activation(
                    ssgn_sb[:], ps2[:], mybir.ActivationFunctionType.Sin, bias=b2s[i][:]
                )
                nc.scalar.activation(
                    c2_sb[:], ps2[:], mybir.ActivationFunctionType.Sin, bias=b2c[i][:]
                )

                # ---- rotate into bundle frame (folded layout)
                swap = wk1.tile([128, 1024], f32, tag="swap", name=f"swap_{i}")
                nc.vector.tensor_copy(swap[0:B, :], h_sb[B:TD, :])
                nc.vector.tensor_copy(swap[B:TD, :], h_sb[0:B, :])
                nc.vector.tensor_copy(swap[TD : TD + B, :], h_sb[TD + B : 128, :])
                nc.vector.tensor_copy(swap[TD + B : 128, :], h_sb[TD : TD + B, :])
                rot = wk1.tile([128, 1024], f32, tag="rot", name=f"rot_{i}")
                nc.vector.tensor_mul(rot[:], c2_sb[:], h_sb[:])
                tmp = wk1.tile([128, 1024], f32, tag="tmp", name=f"tmp_{i}")
                nc.vector.tensor_mul(tmp[:], ssgn_sb[:], swap[:])
                nc.vector.tensor_add(rot[:], rot[:], tmp[:])

                # ---- linear transform H = lt(rot); res = H
                psH = phip.tile([128, 1024], f32, tag="phi", name=f"psH_{i}")
                for n in range(2):
                    sl = slice(n * 512, (n + 1) * 512)
                    nc.tensor.matmul(
                        psH[0:TD, sl], ltw[i][0:TD, :], rot[0:TD, sl],
                        start=True, stop=True, tile_position=(0, 0),
                    )
                    nc.tensor.matmul(
                        psH[TD:128, sl], ltw[i][TD:128, :], rot[TD:128, sl],
                        start=True, stop=True, tile_position=(64, 64),
                    )
                nc.vector.tensor_scalar_add(res_sb[:], psH[:], ltb[i][:])

                # term_0 packs (consumed by step 1); scale = -1/1
                emit_pack_phase(res_sb, -1.0, 0, 0, f"h{i}")
                emit_pack_phase(res_sb, -1.0, 1, 0, f"h{i}b")
                # slot fetches for step 1 (queued behind the H AllGathers)
                tts = fetch_slots(0, f"h{i}")

                # ---- Taylor diffusion
                for k in range(1, K + 1):
                    pc = (k - 1) % 2   # parity consumed
                    pp = k % 2         # parity produced

                    def drain_phase(accs, ph, k=k, i=i):
                        """Pack+AG first (critical path), then res += acc*DSC."""
                        acc = accs[ph]
                        sl = slice(ph * 512, (ph + 1) * 512)
                        pack = None
                        if k < K:
                            tbf = wk2.tile(
                                [128, 512], bf16, tag="tbf", name=f"tb_{i}_{k}_{ph}"
                            )
                            nc.scalar.activation(
                                tbf[:], acc[:],
                                mybir.ActivationFunctionType.Copy,
                                scale=-DSC / (k + 1),
                            )
                            pack = packp.tile(
                                [128, 512], bf16, tag=f"pk{ph}", name=f"pk_{i}_{k}_{ph}"
                            )
                            for half in range(2):
                                t_ps = trp.tile([128, 256], bf16, tag="trp")
                                for b in range(4):
                                    nc.tensor.transpose(
                                        t_ps[:, b * TD : (b + 1) * TD],
                                        tbf[half * TD : (half + 1) * TD,
                                            b * 128 : (b + 1) * 128],
                                        ident2[half * TD : (half + 1) * TD, :],
                                    )
                                nc.vector.tensor_copy(
                                    pack[:, half * 256 : (half + 1) * 256], t_ps[:]
                                )
                            nc.gpsimd.dma_start(out=loc_d[ph][pp][:, :], in_=pack[:])
                            nc.gpsimd.collective_compute(
                                "AllGather",
                                mybir.AluOpType.bypass,
                                replica_groups=RG,
                                ins=[loc_d[ph][pp][:, :]],
                                outs=[full_d[ph][pp][:, :, :]],
                            )
                        stage = wk2.tile(
                            [128, 512], f32, tag="stage", name=f"st_{i}_{k}_{ph}"
                        )
                        nc.scalar.activation(
                            stage[:], acc[:],
                            mybir.ActivationFunctionType.Copy, scale=DSC,
                        )
                        nc.vector.tensor_add(res_sb[:, sl], res_sb[:, sl], stage[:])
                        return pack

                    # streamed L half-units, in exact consumption order
                    stiles = {}
                    for s, (ph, t) in enumerate(STREAM_ORDER):
                        lt = lpool.tile(
                            [128, 4096], fp8, tag="lt", name=f"ls_{i}_{k}_{s}"
                        )
                        nc.sync.dma_start(out=lt[:], in_=Lstr_d[s, :, :])
                        stiles[(ph, t)] = lt
                    accs = [
                        accp.tile([128, 512], f32, tag="acc", name=f"acc_{i}_{k}_0"),
                        accp.tile([128, 512], f32, tag="acc", name=f"acc_{i}_{k}_1"),
                    ]
                    # AG_B-independent work first (both phases)
                    for ph, grp in GROUPS_PRE:
                        emit_group(accs, ph, grp, tts, stiles)
                    # phase A tail -> drain+AG_A early
                    for ph, grp in GROUPS_A_TAIL:
                        emit_group(accs, ph, grp, tts, stiles)
                    drain_phase(accs, 0)
                    # phase B tail -> drain+AG_B
                    for ph, grp in GROUPS_B_TAIL:
                        emit_group(accs, ph, grp, tts, stiles)
                    drain_phase(accs, 1)
                    if k < K:
                        # slot fetches for step k+1: queued on gpsimd behind
                        # this step's AllGathers; they wait on the AG sems,
                        # then all fire across DMA queues in parallel
                        tts = fetch_slots(pp, f"{i}_{k}")

                # ---- rotate back, gelu, residual
                swap2 = wk1.tile([128, 1024], f32, tag="swap", name=f"swap2_{i}")
                nc.vector.tensor_copy(swap2[0:B, :], res_sb[B:TD, :])
                nc.vector.tensor_copy(swap2[B:TD, :], res_sb[0:B, :])
                nc.vector.tensor_copy(swap2[TD : TD + B, :], res_sb[TD + B : 128, :])
                nc.vector.tensor_copy(swap2[TD + B : 128, :], res_sb[TD : TD + B, :])
                rot2 = wk1.tile([128, 1024], f32, tag="rot", name=f"rot2_{i}")
                nc.vector.tensor_mul(rot2[:], c2_sb[:], res_sb[:])
                tmp2 = wk1.tile([128, 1024], f32, tag="tmp", name=f"tmp2_{i}")
                nc.vector.tensor_mul(tmp2[:], ssgn_sb[:], swap2[:])
                nc.vector.tensor_sub(rot2[:], rot2[:], tmp2[:])
                g2 = wk1.tile([128, 1024], f32, tag="g", name=f"g2_{i}")
                nc.scalar.activation(
                    g2[:], rot2[:], mybir.ActivationFunctionType.Gelu
                )
                nc.vector.tensor_add(h_sb[:], h_sb[:], g2[:])

            # ---- output projection (per node-half)
            for half in range(2):
                pso = phip.tile([D_OUT, 1024], f32, tag="phi", name=f"pso_{half}")
                hsl = slice(half * TD, (half + 1) * TD)
                for n in range(2):
                    sl = slice(n * 512, (n + 1) * 512)
                    nc.tensor.matmul(
                        pso[:, sl], outw[hsl, :], h_sb[hsl, sl],
                        start=True, stop=True,
                        tile_position=(64 * half, 0),
                    )
                o_sb = wk1.tile([D_OUT, 1024], f32, tag="g", name=f"o_{half}")
                nc.vector.tensor_scalar_add(o_sb[:], pso[:], outb[:])
                nc.sync.dma_start(
                    out=outT_d[:, half * 1024 : (half + 1) * 1024], in_=o_sb[:]
                )

    nc.compile()
    return nc


def kernel(**inputs):
    x = np.asarray(inputs["x"], dtype=np.float32)
    L = np.asarray(inputs["L"], dtype=np.float32)
    emb_W = np.asarray(inputs["emb_W"], dtype=np.float32)
    emb_b = np.asarray(inputs["emb_b"], dtype=np.float32)
    phi_W1 = np.asarray(inputs["phi_W1"], dtype=np.float32)
    phi_b1 = np.asarray(inputs["phi_b1"], dtype=np.float32)
    phi_W2 = np.asarray(inputs["phi_W2"], dtype=np.float32)
    phi_b2 = np.asarray(inputs["phi_b2"], dtype=np.float32)
    lt_W = np.asarray(inputs["lt_W"], dtype=np.float32)
    lt_b = np.asarray(inputs["lt_b"], dtype=np.float32)
    out_W = np.asarray(inputs["out_W"], dtype=np.float32)
    out_b = np.asarray(inputs["out_b"], dtype=np.float32)

    perm = np.concatenate([np.arange(0, TD, 2), np.arange(1, TD, 2)])

    def stack2(a):  # duplicate along partition axis
        return np.ascontiguousarray(np.concatenate([a, a], axis=0))

    embWt = np.ascontiguousarray(emb_W.T[:, perm])
    embB = stack2(emb_b[perm][:, None])
    w1 = np.ascontiguousarray(
        np.stack([stack2(phi_W1[i].T[perm, :]) for i in range(NL)])
    )
    b1 = np.ascontiguousarray(
        np.stack([stack2(phi_b1[i][:, None]) for i in range(NL)])
    )
    w2 = np.ascontiguousarray(
        np.stack(
            [
                stack2(np.concatenate([-phi_W2[i].T, phi_W2[i].T], axis=1))
                for i in range(NL)
            ]
        )
    )
    b2s = np.ascontiguousarray(
        np.stack(
            [
                stack2(np.concatenate([-phi_b2[i], phi_b2[i]])[:, None])
                for i in range(NL)
            ]
        )
    )
    b2c = (b2s + np.float32(np.pi / 2)).astype(np.float32)
    ltw = np.ascontiguousarray(
        np.stack([stack2(lt_W[i].T[perm][:, perm]) for i in range(NL)])
    )
    ltb = np.ascontiguousarray(
        np.stack([stack2(lt_b[i][perm][:, None]) for i in range(NL)])
    )
    outw = stack2(np.ascontiguousarray(out_W.T[perm, :]))
    outb = np.ascontiguousarray(out_b[:, None])

    # quantize L once: transposed, scaled, e3m4
    L8T = np.ascontiguousarray(L.T * np.float32(LSC)).astype(F8)

    shared = {
        "embWt": embWt, "embB": embB, "w1": w1, "b1": b1, "w2": w2,
        "b2s": b2s, "b2c": b2c, "ltw": ltw, "ltb": ltb,
        "outw": outw, "outb": outb,
    }

    # column selections per phase: (cg0 block | cg1 block)
    colsel = [np.r_[0:512, 1024:1536], np.r_[512:1024, 1536:2048]]

    in_maps = []
    for core in range(M):
        # row order: per unit t = (chunk, rank offset), 512 global rows
        row_order = np.concatenate(
            [j * R + c * 512 + np.arange(512) for (c, j) in ORDER]
        )
        Y = L8T[row_order][:, core * R : (core + 1) * R]  # [16384, 2048]
        Y4 = Y.reshape(32, 4, 128, R)  # (t, jj, p, n)
        tiles = np.empty((32, 2, 128, 4096), dtype=F8)
        for p in range(2):
            Z = Y4[:, :, :, colsel[p]]  # [32, 4, 128, 1024]
            tiles[:, p] = np.transpose(Z, (0, 2, 1, 3)).reshape(32, 128, 4096)
        Lres = np.ascontiguousarray(tiles[:NRES])
        Lstr = np.ascontiguousarray(
            np.stack([tiles[t, ph] for (ph, t) in STREAM_ORDER])
        )
        in_maps.append(
            {
                "xT": np.ascontiguousarray(x[core * R : (core + 1) * R].T),
                "Lres": Lres,
                "Lstr": Lstr,
                **shared,
            }
        )

    if "nc" not in _CACHE:
        _CACHE["nc"] = _build()
    nc = _CACHE["nc"]

    trace = bool(os.environ.get("BUNN_TRACE"))
    if trace:
        _install_ntff_shim()
    res = run_bass_kernel_spmd(nc, in_maps, list(range(M)), trace=trace)
    if trace and res.exec_time_ns is not None:
        print(f"HW exec time: {res.exec_time_ns} ns")
        _CACHE["exec_time_ns"] = res.exec_time_ns

    out = np.empty((N, D_OUT), dtype=np.float32)
    for core in range(M):
        out[core * R : (core + 1) * R, :] = res.results[core]["outT"].T
    return out


# revision 16
# speedup vs baseline: 1.1439x; 1.1175x over previous
"""Trainium2 Bass kernel for BuNN (nn_BuNN_10797547782311).

Row-shard L over 8 cores. Per Taylor step, each core contracts its
[2048, 16384] L block (fp8 e3m4, x128 scale, pre-transposed/pre-tiled on
host) against the replicated bf16 term. Key optimizations over v1:

- L stored as fp8 e3m4 (x128): halves HBM traffic; PE-exact (probed).
- 14 of 32 contraction units resident in SBUF (~112KB/partition):
  per-step streamed DMA drops to 18MB.
- Column-tiled matmul pairs (tile_position (0,0)/(0,64)): two 64-wide
  matmuls run concurrently on both halves of the PE array.
- Two-phase steps: output node-blocks {0,2} complete first, are cast /
  transposed / AllGathered while blocks {1,3} compute -> 2 AGs per step
  whose ~24us latency hides under the opposite phase's matmuls.
- Work tiles folded to [128, 1024]: partitions = td x node-half.

Node-parallel math identical to v1 (td-permuted rotation layout, signed
duplicated angles with sin(x + pi/2) for cos).
"""

import os
import sys
import types

import numpy as np
import ml_dtypes

import concourse.bacc as bacc
import concourse.tile as tile
from concourse import mybir
from concourse.bass_utils import run_bass_kernel_spmd
from concourse.bass import ds
from concourse.masks import make_identity

# Problem config (hardcoded)
N, D_IN, D_OUT = 16384, 128, 40
B = 32
TD = 2 * B          # 64
NL = 4              # layers
K = 8               # Taylor steps
M = 8               # cores
R = N // M          # 2048 rows per core

NRES = 13           # resident contraction units (of 32)
NSTR = 32 - NRES    # streamed units
LSC = 128.0         # L fp8 scale (power of 2)
DSC = 1.0 / LSC     # descale for psum -> term
# pow2 scale exponents for the fp8e3 term packs: pack consumed at step k
# is cast with x2^G[k-1] so its std lands ~1 (e3m4 range +-15.5).
G = [2, 3, 4, 6, 9, 11, 14, 17]

f32 = mybir.dt.float32
bf16 = mybir.dt.bfloat16
fp8 = mybir.dt.float8e3
BF = ml_dtypes.bfloat16
F8 = ml_dtypes.float8_e3m4

# contraction-unit consumption order: (chunk, absolute rank slot j).
# All 8 slots (self included) are consumed from the AllGather output at
# static offsets -- no dynamic-offset DMAs anywhere.
ORDER = [(c, j) for c in (0, 2, 1, 3) for j in range(M)]

# unit-index groups (into ORDER): c0=0..7, c2=8..15, c1=16..23, c3=24..31
GC0 = list(range(0, 8))
GC2 = list(range(8, 16))
GC1 = list(range(16, 24))
GC3 = list(range(24, 32))
# per-step global emission order: ALL of phase A first so AG_A (the
# critical-cycle collective) triggers as early as possible, then phase B.
GROUPS_PRE = [(0, GC0), (0, GC2)]
GROUPS_A_TAIL = [(0, GC1), (0, GC3)]
GROUPS_B_TAIL = [(1, GC0), (1, GC2), (1, GC1), (1, GC3)]
STREAM_ORDER = [
    (ph, t)
    for (ph, grp) in GROUPS_PRE + GROUPS_A_TAIL + GROUPS_B_TAIL
    for t in grp
    if t >= NRES
]

_CACHE = {}


def _install_ntff_shim():
    try:
        from antenv.axon_hooks import get_axon_ntff_profile_hook  # noqa: F401
    except ImportError:
        try:
            from trn_agent_boot.trn_boot import _ntff_profile_via_ctypes

            _hook = _ntff_profile_via_ctypes("/opt/axon/libaxon_pjrt.so")
            _m = types.ModuleType("antenv.axon_hooks")
            _m.get_axon_ntff_profile_hook = lambda: _hook
            _m.set_axon_ntff_profile_hook = lambda h: None
            sys.modules["antenv.axon_hooks"] = _m
        except Exception:
            pass


def _build():
    nc = bacc.Bacc(None, target_bir_lowering=False, debug=False, num_devices=M)

    # ---- per-core inputs (host pre-transformed)
    xT_d = nc.dram_tensor("xT", [D_IN, R], f32, kind="ExternalInput")
    Lres_d = nc.dram_tensor("Lres", [NRES, 2, 128, 4096], fp8, kind="ExternalInput")
    Lstr_d = nc.dram_tensor("Lstr", [2 * NSTR, 128, 4096], fp8, kind="ExternalInput")
    embWt_d = nc.dram_tensor("embWt", [D_IN, TD], f32, kind="ExternalInput")
    embB_d = nc.dram_tensor("embB", [128, 1], f32, kind="ExternalInput")
    w1_d = nc.dram_tensor("w1", [NL, 128, TD], f32, kind="ExternalInput")
    b1_d = nc.dram_tensor("b1", [NL, 128, 1], f32, kind="ExternalInput")
    w2_d = nc.dram_tensor("w2", [NL, 128, TD], f32, kind="ExternalInput")
    b2s_d = nc.dram_tensor("b2s", [NL, 128, 1], f32, kind="ExternalInput")
    b2c_d = nc.dram_tensor("b2c", [NL, 128, 1], f32, kind="ExternalInput")
    ltw_d = nc.dram_tensor("ltw", [NL, 128, TD], f32, kind="ExternalInput")
    ltb_d = nc.dram_tensor("ltb", [NL, 128, 1], f32, kind="ExternalInput")
    outw_d = nc.dram_tensor("outw", [128, D_OUT], f32, kind="ExternalInput")
    outb_d = nc.dram_tensor("outb", [D_OUT, 1], f32, kind="ExternalInput")

    outT_d = nc.dram_tensor("outT", [D_OUT, R], f32, kind="ExternalOutput")

    # collective buffers: per phase (A: chunks 0+2, B: chunks 1+3), parity
    loc_d = [
        [nc.dram_tensor(f"loc{p}_{q}", [128, 512], fp8) for q in range(2)]
        for p in range(2)
    ]
    full_d = [
        [
            nc.dram_tensor(f"full{p}_{q}", [M, 128, 512], fp8, addr_space="Shared")
            for q in range(2)
        ]
        for p in range(2)
    ]
    RG = [list(range(M))]

    with tile.TileContext(nc) as tc:
        with (
            tc.tile_pool(name="resp", bufs=1) as resp,
            tc.tile_pool(name="lpool", bufs=8) as lpool,
            tc.tile_pool(name="ttp", bufs=20) as ttp,
            tc.tile_pool(name="packp", bufs=2) as packp,
            tc.tile_pool(name="acc", bufs=2, space="PSUM") as accp,
            tc.tile_pool(name="phip", bufs=2, space="PSUM") as phip,
            tc.tile_pool(name="trp", bufs=2, space="PSUM") as trp,
            tc.tile_pool(name="wk1", bufs=1) as wk1,
            tc.tile_pool(name="wk2", bufs=2) as wk2,
            tc.tile_pool(name="sg", bufs=1) as sg,
        ):
            # ---- persistent SBUF state
            ident2 = sg.tile([128, TD], bf16)
            make_identity(nc, ident2[0:TD, :])
            make_identity(nc, ident2[TD:128, :])
            h_sb = sg.tile([128, R // 2], f32)
            res_sb = sg.tile([128, R // 2], f32)
            c2_sb = sg.tile([128, R // 2], f32)
            ssgn_sb = sg.tile([128, R // 2], f32)

            # weights resident in SBUF
            embWt = sg.tile([D_IN, TD], f32)
            nc.sync.dma_start(out=embWt[:], in_=embWt_d[:, :])
            embB = sg.tile([128, 1], f32)
            nc.sync.dma_start(out=embB[:], in_=embB_d[:, :])
            w1 = [sg.tile([128, TD], f32, tag=f"w1_{i}", name=f"w1_{i}") for i in range(NL)]
            b1 = [sg.tile([128, 1], f32, tag=f"b1_{i}", name=f"b1_{i}") for i in range(NL)]
            w2 = [sg.tile([128, TD], f32, tag=f"w2_{i}", name=f"w2_{i}") for i in range(NL)]
            b2s = [sg.tile([128, 1], f32, tag=f"b2s_{i}", name=f"b2s_{i}") for i in range(NL)]
            b2c = [sg.tile([128, 1], f32, tag=f"b2c_{i}", name=f"b2c_{i}") for i in range(NL)]
            ltw = [sg.tile([128, TD], f32, tag=f"ltw_{i}", name=f"ltw_{i}") for i in range(NL)]
            ltb = [sg.tile([128, 1], f32, tag=f"ltb_{i}", name=f"ltb_{i}") for i in range(NL)]
            for i in range(NL):
                nc.sync.dma_start(out=w1[i][:], in_=w1_d[i, :, :])
                nc.sync.dma_start(out=b1[i][:], in_=b1_d[i, :, :])
                nc.sync.dma_start(out=w2[i][:], in_=w2_d[i, :, :])
                nc.sync.dma_start(out=b2s[i][:], in_=b2s_d[i, :, :])
                nc.sync.dma_start(out=b2c[i][:], in_=b2c_d[i, :, :])
                nc.sync.dma_start(out=ltw[i][:], in_=ltw_d[i, :, :])
                nc.sync.dma_start(out=ltb[i][:], in_=ltb_d[i, :, :])
            outw = sg.tile([128, D_OUT], f32)
            nc.sync.dma_start(out=outw[:], in_=outw_d[:, :])
            outb = sg.tile([D_OUT, 1], f32)
            nc.sync.dma_start(out=outb[:], in_=outb_d[:, :])

            # resident L units (both phases)
            lres = []
            for t in range(NRES):
                pair = []
                for p in range(2):
                    lt = sg.tile([128, 4096], fp8, tag=f"lr{t}_{p}", name=f"lr{t}_{p}")
                    nc.sync.dma_start(out=lt[:], in_=Lres_d[t, p, :, :])
                    pair.append(lt)
                lres.append(pair)

            # ---- embedding: h = emb(x)  (folded [128, 1024] layout)
            xt1 = lpool.tile([D_IN, R // 2], f32, tag="lt", name="xt1")
            nc.sync.dma_start(out=xt1[:], in_=xT_d[:, 0 : R // 2])
            xt2 = lpool.tile([D_IN, R // 2], f32, tag="lt", name="xt2")
            nc.sync.dma_start(out=xt2[:], in_=xT_d[:, R // 2 : R])
            hps = phip.tile([128, 1024], f32, tag="phi")
            for n in range(2):
                sl = slice(n * 512, (n + 1) * 512)
                nc.tensor.matmul(
                    hps[0:TD, sl], embWt[:], xt1[:, sl],
                    start=True, stop=True, tile_position=(0, 0),
                )
                nc.tensor.matmul(
                    hps[TD:128, sl], embWt[:], xt2[:, sl],
                    start=True, stop=True, tile_position=(0, 64),
                )
            nc.vector.tensor_scalar_add(h_sb[:], hps[:], embB[:])

            def emit_pack_phase(src_sb, scale, ph, par, tag):
                """Cast src[:, ph*512:+512]*scale -> bf16, transpose to
                node-major pack [128, 512], DMA to loc, AllGather."""
                sl = slice(ph * 512, (ph + 1) * 512)
                tbf = wk2.tile([128, 512], bf16, tag="tbf", name=f"tbf_{tag}")
                nc.scalar.activation(
                    tbf[:], src_sb[:, sl],
                    mybir.ActivationFunctionType.Copy, scale=scale,
                )
                pack = packp.tile([128, 512], fp8, tag=f"pk{ph}", name=f"pk_{tag}")
                for half in range(2):
                    t_ps = trp.tile([128, 256], bf16, tag="trp")
                    for b in range(4):
                        nc.tensor.transpose(
                            t_ps[:, b * TD : (b + 1) * TD],
                            tbf[half * TD : (half + 1) * TD, b * 128 : (b + 1) * 128],
                            ident2[half * TD : (half + 1) * TD, :],
                        )
                    nc.vector.tensor_copy(
                        pack[:, half * 256 : (half + 1) * 256], t_ps[:]
                    )
                nc.gpsimd.dma_start(out=loc_d[ph][par][:, :], in_=pack[:])
                nc.gpsimd.collective_compute(
                    "AllGather",
                    mybir.AluOpType.bypass,
                    replica_groups=RG,
                    ins=[loc_d[ph][par][:, :]],
                    outs=[full_d[ph][par][:, :, :]],
                )
                return pack

            def fetch_slots(par, tag):
                """Static per-slot reads of both phase AG buffers (16 DMAs)."""
                tts = {}
                for ph in range(2):
                    for j in range(M):
                        tt = ttp.tile(
                            [128, 512], fp8, tag="tt", name=f"tt_{tag}_{ph}_{j}"
                        )
                        nc.gpsimd.dma_start(
                            out=tt[:], in_=full_d[ph][par][j, :, :]
                        )
                        tts[(ph, j)] = tt
                return tts

            def emit_group(accs, ph, units, tts, stiles):
                """Matmuls for one (phase, unit-group)."""
                acc = accs[ph]
                for t in units:
                    c, j = ORDER[t]
                    cph = 0 if c in (0, 2) else 1
                    coff = 256 if c >= 2 else 0
                    lh_tile = tts[(cph, j)]
                    if t < NRES:
                        rhs = lres[t][ph]
                    else:
                        rhs = stiles[(ph, t)]
                    for jj in range(4):
                        lh = lh_tile[:, coff + jj * TD : coff + (jj + 1) * TD]
                        first = t == 0 and jj == 0
                        last = t == 31 and jj == 3
                        nc.tensor.matmul(
                            acc[0:TD, :],
                            lh,
                            rhs[:, jj * 1024 : jj * 1024 + 512],
                            start=first, stop=last, tile_position=(0, 0),
                        )
                        nc.tensor.matmul(
                            acc[TD:128, :],
                            lh,
                            rhs[:, jj * 1024 + 512 : (jj + 1) * 1024],
                            start=first, stop=last, tile_position=(0, 64),
                        )

            for i in range(NL):
                # ---- phi MLP -> signed duplicated angles -> sin/cos
                ps1 = phip.tile([128, 1024], f32, tag="phi", name=f"ps1_{i}")
                for n in range(2):
                    sl = slice(n * 512, (n + 1) * 512)
                    nc.tensor.matmul(
                        ps1[0:TD, sl], w1[i][0:TD, :], h_sb[0:TD, sl],
                        start=True, stop=True, tile_position=(0, 0),
                    )
                    nc.tensor.matmul(
                        ps1[TD:128, sl], w1[i][TD:128, :], h_sb[TD:128, sl],
                        start=True, stop=True, tile_position=(64, 64),
                    )
                g_sb = wk1.tile([128, 1024], f32, tag="g", name=f"g_{i}")
                nc.scalar.activation(
                    g_sb[:], ps1[:], mybir.ActivationFunctionType.Gelu, bias=b1[i][:]
                )
                ps2 = phip.tile([128, 1024], f32, tag="phi", name=f"ps2_{i}")
                for n in range(2):
                    sl = slice(n * 512, (n + 1) * 512)
                    nc.tensor.matmul(
                        ps2[0:TD, sl], w2[i][0:TD, :], g_sb[0:TD, sl],
                        start=True, stop=True, tile_position=(0, 0),
                    )
                    nc.tensor.matmul(
                        ps2[TD:128, sl], w2[i][TD:128, :], g_sb[TD:128, sl],
                        start=True, stop=True, tile_position=(64, 64),
                    )
                nc.scalar.activation(
                    ssgn_sb[:], ps2[:], mybir.ActivationFunctionType.Sin, bias=b2s[i][:]
                )
                nc.scalar.activation(
                    c2_sb[:], ps2[:], mybir.ActivationFunctionType.Sin, bias=b2c[i][:]
                )

                # ---- rotate into bundle frame (folded layout)
                swap = wk1.tile([128, 1024], f32, tag="swap", name=f"swap_{i}")
                nc.vector.tensor_copy(swap[0:B, :], h_sb[B:TD, :])
                nc.vector.tensor_copy(swap[B:TD, :], h_sb[0:B, :])
                nc.vector.tensor_copy(swap[TD : TD + B, :], h_sb[TD + B : 128, :])
                nc.vector.tensor_copy(swap[TD + B : 128, :], h_sb[TD : TD + B, :])
                rot = wk1.tile([128, 1024], f32, tag="rot", name=f"rot_{i}")
                nc.vector.tensor_mul(rot[:], c2_sb[:], h_sb[:])
                tmp = wk1.tile([128, 1024], f32, tag="tmp", name=f"tmp_{i}")
                nc.vector.tensor_mul(tmp[:], ssgn_sb[:], swap[:])
                nc.vector.tensor_add(rot[:], rot[:], tmp[:])

                # ---- linear transform H = lt(rot); res = H
                psH = phip.tile([128, 1024], f32, tag="phi", name=f"psH_{i}")
                for n in range(2):
                    sl = slice(n * 512, (n + 1) * 512)
                    nc.tensor.matmul(
                        psH[0:TD, sl], ltw[i][0:TD, :], rot[0:TD, sl],
                        start=True, stop=True, tile_position=(0, 0),
                    )
                    nc.tensor.matmul(
                        psH[TD:128, sl], ltw[i][TD:128, :], rot[TD:128, sl],
                        start=True, stop=True, tile_position=(64, 64),
                    )
                nc.vector.tensor_scalar_add(res_sb[:], psH[:], ltb[i][:])

                # term_0 packs (consumed by step 1); scale = -2^G[0]
                emit_pack_phase(res_sb, -float(2.0 ** G[0]), 0, 0, f"h{i}")
                emit_pack_phase(res_sb, -float(2.0 ** G[0]), 1, 0, f"h{i}b")
                # slot fetches for step 1 (queued behind the H AllGathers)
                tts = fetch_slots(0, f"h{i}")

                # ---- Taylor diffusion
                for k in range(1, K + 1):
                    pc = (k - 1) % 2   # parity consumed
                    pp = k % 2         # parity produced

                    def drain_phase(accs, ph, k=k, i=i):
                        """Pack+AG first (critical path), then res += acc*DSC."""
                        acc = accs[ph]
                        sl = slice(ph * 512, (ph + 1) * 512)
                        pack = None
                        if k < K:
                            tbf = wk2.tile(
                                [128, 512], bf16, tag="tbf", name=f"tb_{i}_{k}_{ph}"
                            )
                            nc.scalar.activation(
                                tbf[:], acc[:],
                                mybir.ActivationFunctionType.Copy,
                                scale=-float(2.0 ** (G[k] - 7 - G[k - 1])) / (k + 1),
                            )
                            pack = packp.tile(
                                [128, 512], fp8, tag=f"pk{ph}", name=f"pk_{i}_{k}_{ph}"
                            )
                            for half in range(2):
                                t_ps = trp.tile([128, 256], bf16, tag="trp")
                                for b in range(4):
                                    nc.tensor.transpose(
                                        t_ps[:, b * TD : (b + 1) * TD],
                                        tbf[half * TD : (half + 1) * TD,
                                            b * 128 : (b + 1) * 128],
                                        ident2[half * TD : (half + 1) * TD, :],
                                    )
                                nc.vector.tensor_copy(
                                    pack[:, half * 256 : (half + 1) * 256], t_ps[:]
                                )
                            nc.gpsimd.dma_start(out=loc_d[ph][pp][:, :], in_=pack[:])
                            nc.gpsimd.collective_compute(
                                "AllGather",
                                mybir.AluOpType.bypass,
                                replica_groups=RG,
                                ins=[loc_d[ph][pp][:, :]],
                                outs=[full_d[ph][pp][:, :, :]],
                            )
                        stage = wk2.tile(
                            [128, 512], f32, tag="stage", name=f"st_{i}_{k}_{ph}"
                        )
                        nc.scalar.activation(
                            stage[:], acc[:],
                            mybir.ActivationFunctionType.Copy,
                            scale=float(2.0 ** (-(7 + G[k - 1]))),
                        )
                        nc.vector.tensor_add(res_sb[:, sl], res_sb[:, sl], stage[:])
                        return pack

                    # streamed L half-units, in exact consumption order
                    stiles = {}
                    for s, (ph, t) in enumerate(STREAM_ORDER):
                        lt = lpool.tile(
                            [128, 4096], fp8, tag="lt", name=f"ls_{i}_{k}_{s}"
                        )
                        nc.sync.dma_start(out=lt[:], in_=Lstr_d[s, :, :])
                        stiles[(ph, t)] = lt
                    accs = [
                        accp.tile([128, 512], f32, tag="acc", name=f"acc_{i}_{k}_0"),
                        accp.tile([128, 512], f32, tag="acc", name=f"acc_{i}_{k}_1"),
                    ]
                    # AG_B-independent work first (both phases)
                    for ph, grp in GROUPS_PRE:
                        emit_group(accs, ph, grp, tts, stiles)
                    # phase A tail -> drain+AG_A early
                    for ph, grp in GROUPS_A_TAIL:
                        emit_group(accs, ph, grp, tts, stiles)
                    drain_phase(accs, 0)
                    # phase B tail -> drain+AG_B
                    for ph, grp in GROUPS_B_TAIL:
                        emit_group(accs, ph, grp, tts, stiles)
                    drain_phase(accs, 1)
                    if k < K:
                        # slot fetches for step k+1: queued on gpsimd behind
                        # this step's AllGathers; they wait on the AG sems,
                        # then all fire across DMA queues in parallel
                        tts = fetch_slots(pp, f"{i}_{k}")

                # ---- rotate back, gelu, residual
                swap2 = wk1.tile([128, 1024], f32, tag="swap", name=f"swap2_{i}")
                nc.vector.tensor_copy(swap2[0:B, :], res_sb[B:TD, :])
                nc.vector.tensor_copy(swap2[B:TD, :], res_sb[0:B, :])
                nc.vector.tensor_copy(swap2[TD : TD + B, :], res_sb[TD + B : 128, :])
                nc.vector.tensor_copy(swap2[TD + B : 128, :], res_sb[TD : TD + B, :])
                rot2 = wk1.tile([128, 1024], f32, tag="rot", name=f"rot2_{i}")
                nc.vector.tensor_mul(rot2[:], c2_sb[:], res_sb[:])
                tmp2 = wk1.tile([128, 1024], f32, tag="tmp", name=f"tmp2_{i}")
                nc.vector.tensor_mul(tmp2[:], ssgn_sb[:], swap2[:])
                nc.vector.tensor_sub(rot2[:], rot2[:], tmp2[:])
                g2 = wk1.tile([128, 1024], f32, tag="g", name=f"g2_{i}")
                nc.scalar.activation(
                    g2[:], rot2[:], mybir.ActivationFunctionType.Gelu
                )
                nc.vector.tensor_add(h_sb[:], h_sb[:], g2[:])

            # ---- output projection (per node-half)
            for half in range(2):
                pso = phip.tile([D_OUT, 1024], f32, tag="phi", name=f"pso_{half}")
                hsl = slice(half * TD, (half + 1) * TD)
                for n in range(2):
                    sl = slice(n * 512, (n + 1) * 512)
                    nc.tensor.matmul(
                        pso[:, sl], outw[hsl, :], h_sb[hsl, sl],
                        start=True, stop=True,
                        tile_position=(64 * half, 0),
                    )
                o_sb = wk1.tile([D_OUT, 1024], f32, tag="g", name=f"o_{half}")
                nc.vector.tensor_scalar_add(o_sb[:], pso[:], outb[:])
                nc.sync.dma_start(
                    out=outT_d[:, half * 1024 : (half + 1) * 1024], in_=o_sb[:]
                )

    nc.compile()
    return nc


def kernel(**inputs):
    x = np.asarray(inputs["x"], dtype=np.float32)
    L = np.asarray(inputs["L"], dtype=np.float32)
    emb_W = np.asarray(inputs["emb_W"], dtype=np.float32)
    emb_b = np.asarray(inputs["emb_b"], dtype=np.float32)
    phi_W1 = np.asarray(inputs["phi_W1"], dtype=np.float32)
    phi_b1 = np.asarray(inputs["phi_b1"], dtype=np.float32)
    phi_W2 = np.asarray(inputs["phi_W2"], dtype=np.float32)
    phi_b2 = np.asarray(inputs["phi_b2"], dtype=np.float32)
    lt_W = np.asarray(inputs["lt_W"], dtype=np.float32)
    lt_b = np.asarray(inputs["lt_b"], dtype=np.float32)
    out_W = np.asarray(inputs["out_W"], dtype=np.float32)
    out_b = np.asarray(inputs["out_b"], dtype=np.float32)

    perm = np.concatenate([np.arange(0, TD, 2), np.arange(1, TD, 2)])

    def stack2(a):  # duplicate along partition axis
        return np.ascontiguousarray(np.concatenate([a, a], axis=0))

    embWt = np.ascontiguousarray(emb_W.T[:, perm])
    embB = stack2(emb_b[perm][:, None])
    w1 = np.ascontiguousarray(
        np.stack([stack2(phi_W1[i].T[perm, :]) for i in range(NL)])
    )
    b1 = np.ascontiguousarray(
        np.stack([stack2(phi_b1[i][:, None]) for i in range(NL)])
    )
    w2 = np.ascontiguousarray(
        np.stack(
            [
                stack2(np.concatenate([-phi_W2[i].T, phi_W2[i].T], axis=1))
                for i in range(NL)
            ]
        )
    )
    b2s = np.ascontiguousarray(
        np.stack(
            [
                stack2(np.concatenate([-phi_b2[i], phi_b2[i]])[:, None])
                for i in range(NL)
            ]
        )
    )
    b2c = (b2s + np.float32(np.pi / 2)).astype(np.float32)
    ltw = np.ascontiguousarray(
        np.stack([stack2(lt_W[i].T[perm][:, perm]) for i in range(NL)])
    )
    ltb = np.ascontiguousarray(
        np.stack([stack2(lt_b[i][perm][:, None]) for i in range(NL)])
    )
    outw = stack2(np.ascontiguousarray(out_W.T[perm, :]))
    outb = np.ascontiguousarray(out_b[:, None])

    # quantize L once: transposed, scaled, e3m4
    L8T = np.ascontiguousarray(L.T * np.float32(LSC)).astype(F8)

    shared = {
        "embWt": embWt, "embB": embB, "w1": w1, "b1": b1, "w2": w2,
        "b2s": b2s, "b2c": b2c, "ltw": ltw, "ltb": ltb,
        "outw": outw, "outb": outb,
    }

    # column selections per phase: (cg0 block | cg1 block)
    colsel = [np.r_[0:512, 1024:1536], np.r_[512:1024, 1536:2048]]

    in_maps = []
    for core in range(M):
        # row order: per unit t = (chunk, rank offset), 512 global rows
        row_order = np.concatenate(
            [j * R + c * 512 + np.arange(512) for (c, j) in ORDER]
        )
        Y = L8T[row_order][:, core * R : (core + 1) * R]  # [16384, 2048]
        Y4 = Y.reshape(32, 4, 128, R)  # (t, jj, p, n)
        tiles = np.empty((32, 2, 128, 4096), dtype=F8)
        for p in range(2):
            Z = Y4[:, :, :, colsel[p]]  # [32, 4, 128, 1024]
            tiles[:, p] = np.transpose(Z, (0, 2, 1, 3)).reshape(32, 128, 4096)
        Lres = np.ascontiguousarray(tiles[:NRES])
        Lstr = np.ascontiguousarray(
            np.stack([tiles[t, ph] for (ph, t) in STREAM_ORDER])
        )
        in_maps.append(
            {
                "xT": np.ascontiguousarray(x[core * R : (core + 1) * R].T),
                "Lres": Lres,
                "Lstr": Lstr,
                **shared,
            }
        )

    if "nc" not in _CACHE:
        _CACHE["nc"] = _build()
    nc = _CACHE["nc"]

    trace = bool(os.environ.get("BUNN_TRACE"))
    if trace:
        _install_ntff_shim()
    res = run_bass_kernel_spmd(nc, in_maps, list(range(M)), trace=trace)
    if trace and res.exec_time_ns is not None:
        print(f"HW exec time: {res.exec_time_ns} ns")
        _CACHE["exec_time_ns"] = res.exec_time_ns

    out = np.empty((N, D_OUT), dtype=np.float32)
    for core in range(M):
        out[core * R : (core + 1) * R, :] = res.results[core]["outT"].T
    return out


# revision 17
# speedup vs baseline: 1.2149x; 1.0621x over previous
"""Trainium2 Bass kernel for BuNN (nn_BuNN_10797547782311).

Row-shard L over 8 cores. Per Taylor step, each core contracts its
[2048, 16384] L block (fp8 e3m4, x128 scale, pre-transposed/pre-tiled on
host) against the replicated bf16 term. Key optimizations over v1:

- L stored as fp8 e3m4 (x128): halves HBM traffic; PE-exact (probed).
- 14 of 32 contraction units resident in SBUF (~112KB/partition):
  per-step streamed DMA drops to 18MB.
- Column-tiled matmul pairs (tile_position (0,0)/(0,64)): two 64-wide
  matmuls run concurrently on both halves of the PE array.
- Two-phase steps: output node-blocks {0,2} complete first, are cast /
  transposed / AllGathered while blocks {1,3} compute -> 2 AGs per step
  whose ~24us latency hides under the opposite phase's matmuls.
- Work tiles folded to [128, 1024]: partitions = td x node-half.

Node-parallel math identical to v1 (td-permuted rotation layout, signed
duplicated angles with sin(x + pi/2) for cos).
"""

import os
import sys
import types

import numpy as np
import ml_dtypes

import concourse.bacc as bacc
import concourse.tile as tile
from concourse import mybir
from concourse.bass_utils import run_bass_kernel_spmd
from concourse.bass import ds
from concourse.masks import make_identity

# Problem config (hardcoded)
N, D_IN, D_OUT = 16384, 128, 40
B = 32
TD = 2 * B          # 64
NL = 4              # layers
K = 8               # Taylor steps
M = 8               # cores
R = N // M          # 2048 rows per core

NRES = 13           # resident contraction units (of 32)
NSTR = 32 - NRES    # streamed units
LSC = 128.0         # L fp8 scale (power of 2)
DSC = 1.0 / LSC     # descale for psum -> term
# pow2 scale exponents for the fp8e3 term packs: pack consumed at step k
# is cast with x2^G[k-1] so its std lands ~1 (e3m4 range +-15.5).
G = [2, 3, 4, 6, 9, 11, 14, 17]

f32 = mybir.dt.float32
bf16 = mybir.dt.bfloat16
fp8 = mybir.dt.float8e3
BF = ml_dtypes.bfloat16
F8 = ml_dtypes.float8_e3m4

# contraction-unit consumption order: (chunk, absolute rank slot j).
# All 8 slots (self included) are consumed from the AllGather output at
# static offsets -- no dynamic-offset DMAs anywhere.
ORDER = [(c, j) for c in (0, 2, 1, 3) for j in range(M)]

# unit-index groups (into ORDER): c0=0..7, c2=8..15, c1=16..23, c3=24..31
GC0 = list(range(0, 8))
GC2 = list(range(8, 16))
GC1 = list(range(16, 24))
GC3 = list(range(24, 32))
# per-step global emission order: ALL of phase A first so AG_A (the
# critical-cycle collective) triggers as early as possible, then phase B.
GROUPS_PRE = [(0, GC0), (0, GC2)]
GROUPS_A_TAIL = [(0, GC1), (0, GC3)]
GROUPS_B_TAIL = [(1, GC0), (1, GC2), (1, GC1), (1, GC3)]
STREAM_ORDER = [
    (ph, t)
    for (ph, grp) in GROUPS_PRE + GROUPS_A_TAIL + GROUPS_B_TAIL
    for t in grp
    if t >= NRES
]

_CACHE = {}


def _install_ntff_shim():
    try:
        from antenv.axon_hooks import get_axon_ntff_profile_hook  # noqa: F401
    except ImportError:
        try:
            from trn_agent_boot.trn_boot import _ntff_profile_via_ctypes

            _hook = _ntff_profile_via_ctypes("/opt/axon/libaxon_pjrt.so")
            _m = types.ModuleType("antenv.axon_hooks")
            _m.get_axon_ntff_profile_hook = lambda: _hook
            _m.set_axon_ntff_profile_hook = lambda h: None
            sys.modules["antenv.axon_hooks"] = _m
        except Exception:
            pass


def _build():
    nc = bacc.Bacc(None, target_bir_lowering=False, debug=False, num_devices=M)

    # ---- per-core inputs (host pre-transformed)
    xT_d = nc.dram_tensor("xT", [D_IN, R], f32, kind="ExternalInput")
    Lres_d = nc.dram_tensor("Lres", [NRES, 2, 128, 4096], fp8, kind="ExternalInput")
    Lstr_d = nc.dram_tensor("Lstr", [2 * NSTR, 128, 4096], fp8, kind="ExternalInput")
    embWt_d = nc.dram_tensor("embWt", [D_IN, TD], f32, kind="ExternalInput")
    embB_d = nc.dram_tensor("embB", [128, 1], f32, kind="ExternalInput")
    w1_d = nc.dram_tensor("w1", [NL, 128, TD], f32, kind="ExternalInput")
    b1_d = nc.dram_tensor("b1", [NL, 128, 1], f32, kind="ExternalInput")
    w2_d = nc.dram_tensor("w2", [NL, 128, TD], f32, kind="ExternalInput")
    b2s_d = nc.dram_tensor("b2s", [NL, 128, 1], f32, kind="ExternalInput")
    b2c_d = nc.dram_tensor("b2c", [NL, 128, 1], f32, kind="ExternalInput")
    ltw_d = nc.dram_tensor("ltw", [NL, 128, TD], f32, kind="ExternalInput")
    ltb_d = nc.dram_tensor("ltb", [NL, 128, 1], f32, kind="ExternalInput")
    outw_d = nc.dram_tensor("outw", [128, D_OUT], f32, kind="ExternalInput")
    outb_d = nc.dram_tensor("outb", [D_OUT, 1], f32, kind="ExternalInput")

    outT_d = nc.dram_tensor("outT", [D_OUT, R], f32, kind="ExternalOutput")

    # collective buffers: per phase (A: chunks 0+2, B: chunks 1+3), parity
    loc_d = [
        [nc.dram_tensor(f"loc{p}_{q}", [128, 512], fp8) for q in range(2)]
        for p in range(2)
    ]
    full_d = [
        [
            nc.dram_tensor(f"full{p}_{q}", [M, 128, 512], fp8, addr_space="Shared")
            for q in range(2)
        ]
        for p in range(2)
    ]
    RG = [list(range(M))]

    with tile.TileContext(nc) as tc:
        with (
            tc.tile_pool(name="resp", bufs=1) as resp,
            tc.tile_pool(name="lpool", bufs=12) as lpool,
            tc.tile_pool(name="ttp", bufs=20) as ttp,
            tc.tile_pool(name="packp", bufs=2) as packp,
            tc.tile_pool(name="acc", bufs=2, space="PSUM") as accp,
            tc.tile_pool(name="phip", bufs=2, space="PSUM") as phip,
            tc.tile_pool(name="trp", bufs=2, space="PSUM") as trp,
            tc.tile_pool(name="wk1", bufs=1) as wk1,
            tc.tile_pool(name="wk2", bufs=2) as wk2,
            tc.tile_pool(name="sg", bufs=1) as sg,
        ):
            # ---- persistent SBUF state
            ident2 = sg.tile([128, TD], bf16)
            make_identity(nc, ident2[0:TD, :])
            make_identity(nc, ident2[TD:128, :])
            h_sb = sg.tile([128, R // 2], f32)
            res_sb = sg.tile([128, R // 2], f32)
            c2_sb = sg.tile([128, R // 2], f32)
            ssgn_sb = sg.tile([128, R // 2], f32)

            # weights resident in SBUF
            embWt = sg.tile([D_IN, TD], f32)
            nc.sync.dma_start(out=embWt[:], in_=embWt_d[:, :])
            embB = sg.tile([128, 1], f32)
            nc.sync.dma_start(out=embB[:], in_=embB_d[:, :])
            w1 = [sg.tile([128, TD], f32, tag=f"w1_{i}", name=f"w1_{i}") for i in range(NL)]
            b1 = [sg.tile([128, 1], f32, tag=f"b1_{i}", name=f"b1_{i}") for i in range(NL)]
            w2 = [sg.tile([128, TD], f32, tag=f"w2_{i}", name=f"w2_{i}") for i in range(NL)]
            b2s = [sg.tile([128, 1], f32, tag=f"b2s_{i}", name=f"b2s_{i}") for i in range(NL)]
            b2c = [sg.tile([128, 1], f32, tag=f"b2c_{i}", name=f"b2c_{i}") for i in range(NL)]
            ltw = [sg.tile([128, TD], f32, tag=f"ltw_{i}", name=f"ltw_{i}") for i in range(NL)]
            ltb = [sg.tile([128, 1], f32, tag=f"ltb_{i}", name=f"ltb_{i}") for i in range(NL)]
            for i in range(NL):
                nc.sync.dma_start(out=w1[i][:], in_=w1_d[i, :, :])
                nc.sync.dma_start(out=b1[i][:], in_=b1_d[i, :, :])
                nc.sync.dma_start(out=w2[i][:], in_=w2_d[i, :, :])
                nc.sync.dma_start(out=b2s[i][:], in_=b2s_d[i, :, :])
                nc.sync.dma_start(out=b2c[i][:], in_=b2c_d[i, :, :])
                nc.sync.dma_start(out=ltw[i][:], in_=ltw_d[i, :, :])
                nc.sync.dma_start(out=ltb[i][:], in_=ltb_d[i, :, :])
            outw = sg.tile([128, D_OUT], f32)
            nc.sync.dma_start(out=outw[:], in_=outw_d[:, :])
            outb = sg.tile([D_OUT, 1], f32)
            nc.sync.dma_start(out=outb[:], in_=outb_d[:, :])

            # ---- embedding: h = emb(x)  (folded [128, 1024] layout)
            xt1 = lpool.tile([D_IN, R // 2], f32, tag="lt", name="xt1")
            nc.sync.dma_start(out=xt1[:], in_=xT_d[:, 0 : R // 2])
            xt2 = lpool.tile([D_IN, R // 2], f32, tag="lt", name="xt2")
            nc.sync.dma_start(out=xt2[:], in_=xT_d[:, R // 2 : R])
            hps = phip.tile([128, 1024], f32, tag="phi")
            for n in range(2):
                sl = slice(n * 512, (n + 1) * 512)
                nc.tensor.matmul(
                    hps[0:TD, sl], embWt[:], xt1[:, sl],
                    start=True, stop=True, tile_position=(0, 0),
                )
                nc.tensor.matmul(
                    hps[TD:128, sl], embWt[:], xt2[:, sl],
                    start=True, stop=True, tile_position=(0, 64),
                )
            nc.vector.tensor_scalar_add(h_sb[:], hps[:], embB[:])

            # resident L units (both phases) -- loaded while layer 0's phi
            # and H computation run
            lres = []
            for t in range(NRES):
                pair = []
                for p in range(2):
                    lt = sg.tile([128, 4096], fp8, tag=f"lr{t}_{p}", name=f"lr{t}_{p}")
                    nc.sync.dma_start(out=lt[:], in_=Lres_d[t, p, :, :])
                    pair.append(lt)
                lres.append(pair)

            def emit_pack_phase(src_sb, scale, ph, par, tag):
                """Cast src[:, ph*512:+512]*scale -> bf16, transpose to
                node-major pack [128, 512], DMA to loc, AllGather."""
                sl = slice(ph * 512, (ph + 1) * 512)
                tbf = wk2.tile([128, 512], bf16, tag="tbf", name=f"tbf_{tag}")
                nc.scalar.activation(
                    tbf[:], src_sb[:, sl],
                    mybir.ActivationFunctionType.Copy, scale=scale,
                )
                pack = packp.tile([128, 512], fp8, tag=f"pk{ph}", name=f"pk_{tag}")
                for half in range(2):
                    t_ps = trp.tile([128, 256], bf16, tag="trp")
                    for b in range(4):
                        nc.tensor.transpose(
                            t_ps[:, b * TD : (b + 1) * TD],
                            tbf[half * TD : (half + 1) * TD, b * 128 : (b + 1) * 128],
                            ident2[half * TD : (half + 1) * TD, :],
                        )
                    nc.vector.tensor_copy(
                        pack[:, half * 256 : (half + 1) * 256], t_ps[:]
                    )
                nc.gpsimd.dma_start(out=loc_d[ph][par][:, :], in_=pack[:])
                nc.gpsimd.collective_compute(
                    "AllGather",
                    mybir.AluOpType.bypass,
                    replica_groups=RG,
                    ins=[loc_d[ph][par][:, :]],
                    outs=[full_d[ph][par][:, :, :]],
                )
                return pack

            def fetch_slots(par, tag):
                """Static per-slot reads of both phase AG buffers (16 DMAs)."""
                tts = {}
                for ph in range(2):
                    for j in range(M):
                        tt = ttp.tile(
                            [128, 512], fp8, tag="tt", name=f"tt_{tag}_{ph}_{j}"
                        )
                        nc.gpsimd.dma_start(
                            out=tt[:], in_=full_d[ph][par][j, :, :]
                        )
                        tts[(ph, j)] = tt
                return tts

            def emit_group(accs, ph, units, tts, stiles):
                """Matmuls for one (phase, unit-group)."""
                acc = accs[ph]
                for t in units:
                    c, j = ORDER[t]
                    cph = 0 if c in (0, 2) else 1
                    coff = 256 if c >= 2 else 0
                    lh_tile = tts[(cph, j)]
                    if t < NRES:
                        rhs = lres[t][ph]
                    else:
                        rhs = stiles[(ph, t)]
                    for jj in range(4):
                        lh = lh_tile[:, coff + jj * TD : coff + (jj + 1) * TD]
                        first = t == 0 and jj == 0
                        last = t == 31 and jj == 3
                        nc.tensor.matmul(
                            acc[0:TD, :],
                            lh,
                            rhs[:, jj * 1024 : jj * 1024 + 512],
                            start=first, stop=last, tile_position=(0, 0),
                        )
                        nc.tensor.matmul(
                            acc[TD:128, :],
                            lh,
                            rhs[:, jj * 1024 + 512 : (jj + 1) * 1024],
                            start=first, stop=last, tile_position=(0, 64),
                        )

            for i in range(NL):
                # ---- phi MLP -> signed duplicated angles -> sin/cos
                ps1 = phip.tile([128, 1024], f32, tag="phi", name=f"ps1_{i}")
                for n in range(2):
                    sl = slice(n * 512, (n + 1) * 512)
                    nc.tensor.matmul(
                        ps1[0:TD, sl], w1[i][0:TD, :], h_sb[0:TD, sl],
                        start=True, stop=True, tile_position=(0, 0),
                    )
                    nc.tensor.matmul(
                        ps1[TD:128, sl], w1[i][TD:128, :], h_sb[TD:128, sl],
                        start=True, stop=True, tile_position=(64, 64),
                    )
                g_sb = wk1.tile([128, 1024], f32, tag="g", name=f"g_{i}")
                nc.scalar.activation(
                    g_sb[:], ps1[:], mybir.ActivationFunctionType.Gelu, bias=b1[i][:]
                )
                ps2 = phip.tile([128, 1024], f32, tag="phi", name=f"ps2_{i}")
                for n in range(2):
                    sl = slice(n * 512, (n + 1) * 512)
                    nc.tensor.matmul(
                        ps2[0:TD, sl], w2[i][0:TD, :], g_sb[0:TD, sl],
                        start=True, stop=True, tile_position=(0, 0),
                    )
                    nc.tensor.matmul(
                        ps2[TD:128, sl], w2[i][TD:128, :], g_sb[TD:128, sl],
                        start=True, stop=True, tile_position=(64, 64),
                    )
                nc.scalar.activation(
                    ssgn_sb[:], ps2[:], mybir.ActivationFunctionType.Sin, bias=b2s[i][:]
                )
                nc.scalar.activation(
                    c2_sb[:], ps2[:], mybir.ActivationFunctionType.Sin, bias=b2c[i][:]
                )

                # ---- rotate into bundle frame (folded layout)
                swap = wk1.tile([128, 1024], f32, tag="swap", name=f"swap_{i}")
                nc.vector.tensor_copy(swap[0:B, :], h_sb[B:TD, :])
                nc.vector.tensor_copy(swap[B:TD, :], h_sb[0:B, :])
                nc.vector.tensor_copy(swap[TD : TD + B, :], h_sb[TD + B : 128, :])
                nc.vector.tensor_copy(swap[TD + B : 128, :], h_sb[TD : TD + B, :])
                rot = wk1.tile([128, 1024], f32, tag="rot", name=f"rot_{i}")
                nc.vector.tensor_mul(rot[:], c2_sb[:], h_sb[:])
                tmp = wk1.tile([128, 1024], f32, tag="tmp", name=f"tmp_{i}")
                nc.vector.tensor_mul(tmp[:], ssgn_sb[:], swap[:])
                nc.vector.tensor_add(rot[:], rot[:], tmp[:])

                # ---- linear transform H = lt(rot); res = H
                psH = phip.tile([128, 1024], f32, tag="phi", name=f"psH_{i}")
                for n in range(2):
                    sl = slice(n * 512, (n + 1) * 512)
                    nc.tensor.matmul(
                        psH[0:TD, sl], ltw[i][0:TD, :], rot[0:TD, sl],
                        start=True, stop=True, tile_position=(0, 0),
                    )
                    nc.tensor.matmul(
                        psH[TD:128, sl], ltw[i][TD:128, :], rot[TD:128, sl],
                        start=True, stop=True, tile_position=(64, 64),
                    )
                nc.vector.tensor_scalar_add(res_sb[:], psH[:], ltb[i][:])

                # term_0 packs (consumed by step 1); scale = -2^G[0]
                emit_pack_phase(res_sb, -float(2.0 ** G[0]), 0, 0, f"h{i}")
                emit_pack_phase(res_sb, -float(2.0 ** G[0]), 1, 0, f"h{i}b")
                # slot fetches for step 1 (queued behind the H AllGathers)
                tts = fetch_slots(0, f"h{i}")

                # ---- Taylor diffusion
                for k in range(1, K + 1):
                    pc = (k - 1) % 2   # parity consumed
                    pp = k % 2         # parity produced

                    def drain_phase(accs, ph, k=k, i=i):
                        """Pack+AG first (critical path), then res += acc*DSC."""
                        acc = accs[ph]
                        sl = slice(ph * 512, (ph + 1) * 512)
                        pack = None
                        if k < K:
                            tbf = wk2.tile(
                                [128, 512], bf16, tag="tbf", name=f"tb_{i}_{k}_{ph}"
                            )
                            nc.scalar.activation(
                                tbf[:], acc[:],
                                mybir.ActivationFunctionType.Copy,
                                scale=-float(2.0 ** (G[k] - 7 - G[k - 1])) / (k + 1),
                            )
                            pack = packp.tile(
                                [128, 512], fp8, tag=f"pk{ph}", name=f"pk_{i}_{k}_{ph}"
                            )
                            for half in range(2):
                                t_ps = trp.tile([128, 256], bf16, tag="trp")
                                for b in range(4):
                                    nc.tensor.transpose(
                                        t_ps[:, b * TD : (b + 1) * TD],
                                        tbf[half * TD : (half + 1) * TD,
                                            b * 128 : (b + 1) * 128],
                                        ident2[half * TD : (half + 1) * TD, :],
                                    )
                                nc.vector.tensor_copy(
                                    pack[:, half * 256 : (half + 1) * 256], t_ps[:]
                                )
                            nc.gpsimd.dma_start(out=loc_d[ph][pp][:, :], in_=pack[:])
                            nc.gpsimd.collective_compute(
                                "AllGather",
                                mybir.AluOpType.bypass,
                                replica_groups=RG,
                                ins=[loc_d[ph][pp][:, :]],
                                outs=[full_d[ph][pp][:, :, :]],
                            )
                        stage = wk2.tile(
                            [128, 512], f32, tag="stage", name=f"st_{i}_{k}_{ph}"
                        )
                        nc.scalar.activation(
                            stage[:], acc[:],
                            mybir.ActivationFunctionType.Copy,
                            scale=float(2.0 ** (-(7 + G[k - 1]))),
                        )
                        nc.vector.tensor_add(res_sb[:, sl], res_sb[:, sl], stage[:])
                        return pack

                    # streamed L half-units, in exact consumption order
                    stiles = {}
                    for s, (ph, t) in enumerate(STREAM_ORDER):
                        lt = lpool.tile(
                            [128, 4096], fp8, tag="lt", name=f"ls_{i}_{k}_{s}"
                        )
                        nc.sync.dma_start(out=lt[:], in_=Lstr_d[s, :, :])
                        stiles[(ph, t)] = lt
                    accs = [
                        accp.tile([128, 512], f32, tag="acc", name=f"acc_{i}_{k}_0"),
                        accp.tile([128, 512], f32, tag="acc", name=f"acc_{i}_{k}_1"),
                    ]
                    # AG_B-independent work first (both phases)
                    for ph, grp in GROUPS_PRE:
                        emit_group(accs, ph, grp, tts, stiles)
                    # phase A tail -> drain+AG_A early
                    for ph, grp in GROUPS_A_TAIL:
                        emit_group(accs, ph, grp, tts, stiles)
                    drain_phase(accs, 0)
                    # phase B tail -> drain+AG_B
                    for ph, grp in GROUPS_B_TAIL:
                        emit_group(accs, ph, grp, tts, stiles)
                    drain_phase(accs, 1)
                    if k < K:
                        # slot fetches for step k+1: queued on gpsimd behind
                        # this step's AllGathers; they wait on the AG sems,
                        # then all fire across DMA queues in parallel
                        tts = fetch_slots(pp, f"{i}_{k}")

                # ---- rotate back, gelu, residual
                swap2 = wk1.tile([128, 1024], f32, tag="swap", name=f"swap2_{i}")
                nc.vector.tensor_copy(swap2[0:B, :], res_sb[B:TD, :])
                nc.vector.tensor_copy(swap2[B:TD, :], res_sb[0:B, :])
                nc.vector.tensor_copy(swap2[TD : TD + B, :], res_sb[TD + B : 128, :])
                nc.vector.tensor_copy(swap2[TD + B : 128, :], res_sb[TD : TD + B, :])
                rot2 = wk1.tile([128, 1024], f32, tag="rot", name=f"rot2_{i}")
                nc.vector.tensor_mul(rot2[:], c2_sb[:], res_sb[:])
                tmp2 = wk1.tile([128, 1024], f32, tag="tmp", name=f"tmp2_{i}")
                nc.vector.tensor_mul(tmp2[:], ssgn_sb[:], swap2[:])
                nc.vector.tensor_sub(rot2[:], rot2[:], tmp2[:])
                g2 = wk1.tile([128, 1024], f32, tag="g", name=f"g2_{i}")
                nc.scalar.activation(
                    g2[:], rot2[:], mybir.ActivationFunctionType.Gelu
                )
                nc.vector.tensor_add(h_sb[:], h_sb[:], g2[:])

            # ---- output projection (per node-half)
            for half in range(2):
                pso = phip.tile([D_OUT, 1024], f32, tag="phi", name=f"pso_{half}")
                hsl = slice(half * TD, (half + 1) * TD)
                for n in range(2):
                    sl = slice(n * 512, (n + 1) * 512)
                    nc.tensor.matmul(
                        pso[:, sl], outw[hsl, :], h_sb[hsl, sl],
                        start=True, stop=True,
                        tile_position=(64 * half, 0),
                    )
                o_sb = wk1.tile([D_OUT, 1024], f32, tag="g", name=f"o_{half}")
                nc.vector.tensor_scalar_add(o_sb[:], pso[:], outb[:])
                nc.sync.dma_start(
                    out=outT_d[:, half * 1024 : (half + 1) * 1024], in_=o_sb[:]
                )

    nc.compile()
    return nc


def kernel(**inputs):
    x = np.asarray(inputs["x"], dtype=np.float32)
    L = np.asarray(inputs["L"], dtype=np.float32)
    emb_W = np.asarray(inputs["emb_W"], dtype=np.float32)
    emb_b = np.asarray(inputs["emb_b"], dtype=np.float32)
    phi_W1 = np.asarray(inputs["phi_W1"], dtype=np.float32)
    phi_b1 = np.asarray(inputs["phi_b1"], dtype=np.float32)
    phi_W2 = np.asarray(inputs["phi_W2"], dtype=np.float32)
    phi_b2 = np.asarray(inputs["phi_b2"], dtype=np.float32)
    lt_W = np.asarray(inputs["lt_W"], dtype=np.float32)
    lt_b = np.asarray(inputs["lt_b"], dtype=np.float32)
    out_W = np.asarray(inputs["out_W"], dtype=np.float32)
    out_b = np.asarray(inputs["out_b"], dtype=np.float32)

    perm = np.concatenate([np.arange(0, TD, 2), np.arange(1, TD, 2)])

    def stack2(a):  # duplicate along partition axis
        return np.ascontiguousarray(np.concatenate([a, a], axis=0))

    embWt = np.ascontiguousarray(emb_W.T[:, perm])
    embB = stack2(emb_b[perm][:, None])
    w1 = np.ascontiguousarray(
        np.stack([stack2(phi_W1[i].T[perm, :]) for i in range(NL)])
    )
    b1 = np.ascontiguousarray(
        np.stack([stack2(phi_b1[i][:, None]) for i in range(NL)])
    )
    w2 = np.ascontiguousarray(
        np.stack(
            [
                stack2(np.concatenate([-phi_W2[i].T, phi_W2[i].T], axis=1))
                for i in range(NL)
            ]
        )
    )
    b2s = np.ascontiguousarray(
        np.stack(
            [
                stack2(np.concatenate([-phi_b2[i], phi_b2[i]])[:, None])
                for i in range(NL)
            ]
        )
    )
    b2c = (b2s + np.float32(np.pi / 2)).astype(np.float32)
    ltw = np.ascontiguousarray(
        np.stack([stack2(lt_W[i].T[perm][:, perm]) for i in range(NL)])
    )
    ltb = np.ascontiguousarray(
        np.stack([stack2(lt_b[i][perm][:, None]) for i in range(NL)])
    )
    outw = stack2(np.ascontiguousarray(out_W.T[perm, :]))
    outb = np.ascontiguousarray(out_b[:, None])

    # quantize L once: transposed, scaled, e3m4
    L8T = np.ascontiguousarray(L.T * np.float32(LSC)).astype(F8)

    shared = {
        "embWt": embWt, "embB": embB, "w1": w1, "b1": b1, "w2": w2,
        "b2s": b2s, "b2c": b2c, "ltw": ltw, "ltb": ltb,
        "outw": outw, "outb": outb,
    }

    # column selections per phase: (cg0 block | cg1 block)
    colsel = [np.r_[0:512, 1024:1536], np.r_[512:1024, 1536:2048]]

    in_maps = []
    for core in range(M):
        # row order: per unit t = (chunk, rank offset), 512 global rows
        row_order = np.concatenate(
            [j * R + c * 512 + np.arange(512) for (c, j) in ORDER]
        )
        Y = L8T[row_order][:, core * R : (core + 1) * R]  # [16384, 2048]
        Y4 = Y.reshape(32, 4, 128, R)  # (t, jj, p, n)
        tiles = np.empty((32, 2, 128, 4096), dtype=F8)
        for p in range(2):
            Z = Y4[:, :, :, colsel[p]]  # [32, 4, 128, 1024]
            tiles[:, p] = np.transpose(Z, (0, 2, 1, 3)).reshape(32, 128, 4096)
        Lres = np.ascontiguousarray(tiles[:NRES])
        Lstr = np.ascontiguousarray(
            np.stack([tiles[t, ph] for (ph, t) in STREAM_ORDER])
        )
        in_maps.append(
            {
                "xT": np.ascontiguousarray(x[core * R : (core + 1) * R].T),
                "Lres": Lres,
                "Lstr": Lstr,
                **shared,
            }
        )

    if "nc" not in _CACHE:
        _CACHE["nc"] = _build()
    nc = _CACHE["nc"]

    trace = bool(os.environ.get("BUNN_TRACE"))
    if trace:
        _install_ntff_shim()
    res = run_bass_kernel_spmd(nc, in_maps, list(range(M)), trace=trace)
    if trace and res.exec_time_ns is not None:
        print(f"HW exec time: {res.exec_time_ns} ns")
        _CACHE["exec_time_ns"] = res.exec_time_ns

    out = np.empty((N, D_OUT), dtype=np.float32)
    for core in range(M):
        out[core * R : (core + 1) * R, :] = res.results[core]["outT"].T
    return out
